# revision 19
# baseline (speedup 1.0000x reference)
"""Transformer block (LN -> MHA -> residual -> LN -> MLP -> residual) on 8 TRN2
NeuronCores.

Sharding: pure row data-parallelism over (batch, sequence-half). Core c handles
batch b = c//2 and query rows [h*512, (h+1)*512) with h = c%2. Each core
computes K/V projections for its full batch locally (small duplicated work),
which removes every cross-core collective. Host reorders each core's batch rows
"own rows first" so the same SPMD program works on all cores; mask columns are
permuted identically (softmax/attention are permutation-invariant over keys).

v2: overlap restructure + fp8 attention.
  - All attention matmuls (Q/K/V proj, scores, AV, O proj) run in fp8-e4m3
    with perf_mode=DoubleRow (two 128-row contraction tiles per issue),
    ~1.8x the bf16 matmul rate. MLP matmuls stay bf16 (fp8 there blows the
    2e-2 error budget; attention-fp8 costs ~1.3e-2, verified by simulation).
  - LN1 runs per-row-tile; V projections for a token tile are emitted right
    after its transpose, so the PE has work ~10us in instead of idling ~80us.
  - softmax drops the max-subtraction: logits are bounded (|q.k|*isq <~ 15,
    masked lanes are -30000 -> exp==0), so exp never overflows fp32/bf16.
  - LN2 is fused into the O-projection loop per query tile; the attention
    residual r stays in SBUF (no DRAM bounce).
  - output stores stream per (qt, fc) chunk.
fp32 PSUM accumulation everywhere; statistics stay fp32.
"""

import numpy as np
import ml_dtypes

import concourse.bass as bass
import concourse.tile as tile
from concourse import bacc, mybir
from concourse.bass_utils import run_bass_kernel_spmd

BF16 = mybir.dt.bfloat16
F32 = mybir.dt.float32
FP8 = mybir.dt.float8e4
AX = mybir.AxisListType
OP = mybir.AluOpType
ACT = mybir.ActivationFunctionType
DR = mybir.MatmulPerfMode.DoubleRow

P = 128
B, T, C, H = 4, 1024, 2048, 4
DH = C // H                      # 512
F = 4 * C                        # 8192
R = T // 2                       # 512 own query rows per core
RT, TT, CT, FT = R // P, T // P, C // P, F // P   # 4, 8, 16, 64
CP = CT // 2                     # 8 double-row contraction steps over C
HT = DH // P                     # 4 feature tiles per head
EPS = 1e-5
ISQ = 1.0 / float(np.sqrt(DH))
NEGBIG = 30000.0


def _bcast_load(nc, pool, dram_ap, name, dtype):
    """Broadcast a [n] DRAM vector to all 128 partitions -> [128, n]."""
    t = pool.tile([P, dram_ap.shape[0]], dtype, name=name)
    src = bass.AP(
        tensor=dram_ap.tensor, offset=dram_ap.offset, ap=[[0, P]] + list(dram_ap.ap)
    )
    nc.gpsimd.dma_start(out=t[:], in_=src)
    return t


def _ln_tile(nc, pool, x_sl, w_bc, b_bc, eps_t, tag, i):
    """LayerNorm one [128, C] tile -> bf16 [128, C] tile."""
    stats = pool.tile([P, 4, 6], F32, name=f"{tag}_stats{i}", tag=f"{tag}_stats",
                      bufs=2)
    for sg in range(4):
        nc.vector.bn_stats(out=stats[:, sg, :], in_=x_sl[:, sg * 512:(sg + 1) * 512])
    mv = pool.tile([P, 2], F32, name=f"{tag}_mv{i}", tag=f"{tag}_mv", bufs=2)
    nc.vector.bn_aggr(out=mv[:], in_=stats[:])
    std = pool.tile([P, 1], F32, name=f"{tag}_std{i}", tag=f"{tag}_std", bufs=2)
    nc.scalar.activation(out=std[:], in_=mv[:, 1:2], func=ACT.Sqrt,
                         bias=eps_t[:], scale=1.0)
    rstd = pool.tile([P, 1], F32, name=f"{tag}_rstd{i}", tag=f"{tag}_rstd", bufs=2)
    nc.vector.reciprocal(rstd[:], std[:])
    nmr = pool.tile([P, 1], F32, name=f"{tag}_nmr{i}", tag=f"{tag}_nmr", bufs=2)
    nc.vector.tensor_scalar(nmr[:], mv[:, 0:1], rstd[:], -1.0, OP.mult, OP.mult)
    xh = pool.tile([P, C], BF16, name=f"{tag}_xh{i}", tag=f"{tag}_xh", bufs=2)
    nc.scalar.activation(out=xh[:], in_=x_sl, func=ACT.Identity,
                         bias=nmr[:], scale=rstd[:])
    xn_t = pool.tile([P, C], BF16, name=f"{tag}_xn{i}", tag=f"{tag}_xn", bufs=2)
    nc.vector.tensor_tensor(xn_t[:], xh[:], w_bc[:], OP.mult)
    nc.vector.tensor_tensor(xn_t[:], xn_t[:], b_bc[:], OP.add)
    return xn_t


def _body(tc):
    nc = tc.nc
    d = {n: nc.dram_tensor(n, s, dt, kind=k).ap() for n, s, dt, k in [
        ("xb", [T, C], BF16, "ExternalInput"),
        ("xq", [R, C], F32, "ExternalInput"),
        ("mask", [R, T], BF16, "ExternalInput"),
        ("wq", [CT, P, CT, P], FP8, "ExternalInput"),
        ("wk", [CT, P, CT, P], FP8, "ExternalInput"),
        ("wv", [P, CT, C], FP8, "ExternalInput"),
        ("wo", [P, CT, 4, 512], FP8, "ExternalInput"),
        ("w1", [FT // 2, P, CT, 2, P], BF16, "ExternalInput"),
        ("w2", [4, FT // 8, P, 8, 512], BF16, "ExternalInput"),
        ("bq", [P, CT], F32, "ExternalInput"),
        ("bk", [P, CT], F32, "ExternalInput"),
        ("b1", [P, FT], F32, "ExternalInput"),
        ("bv", [C], BF16, "ExternalInput"),
        ("bo", [C], BF16, "ExternalInput"),
        ("b2", [C], BF16, "ExternalInput"),
        ("l1w", [C], BF16, "ExternalInput"),
        ("l1b", [C], BF16, "ExternalInput"),
        ("l2w", [C], BF16, "ExternalInput"),
        ("l2b", [C], BF16, "ExternalInput"),
        ("out", [R, C], F32, "ExternalOutput"),
    ]}

    consts = tc.alloc_tile_pool(name="consts", bufs=1)
    eps_t = consts.tile([P, 1], F32, name="eps")
    nc.vector.memset(eps_t[:], EPS)
    p_rd = tc.alloc_tile_pool(name="p_rd", bufs=1, space="DRAM")
    r_d = p_rd.tile([R, C], F32, name="r_d")
    # long-lived SBUF tensors (allocated first: the pool stack is LIFO)
    p_yT = tc.alloc_tile_pool(name="p_yT", bufs=1)
    yT = p_yT.tile([P, CT, R], FP8, name="yT")
    p_wo = tc.alloc_tile_pool(name="p_wo", bufs=1)
    wo_t = p_wo.tile([P, CT, 4, 512], FP8, name="wo_t")
    p_xnT = tc.alloc_tile_pool(name="p_xnT", bufs=1)
    xnT8_lo = p_xnT.tile([P, CT, R], FP8, name="xnT8_lo")
    xnT8_hi = p_xnT.tile([P, CT, R], FP8, name="xnT8_hi")
    xnT8 = [xnT8_lo, xnT8_hi]
    p_vh = tc.alloc_tile_pool(name="p_vh", bufs=1)
    vh = p_vh.tile([P, TT, C], FP8, name="vh")

    # ---------------- Stage A: per-tile LN1 -> transpose -> fp8 cast -> V proj
    p_wv = tc.alloc_tile_pool(name="p_wv", bufs=1)
    wv_t = p_wv.tile([P, CT, C], FP8, name="wv_t")
    lnA = tc.alloc_tile_pool(name="lnA", bufs=1)
    l1w_bc = _bcast_load(nc, lnA, d["l1w"], "l1w_bc", BF16)
    l1b_bc = _bcast_load(nc, lnA, d["l1b"], "l1b_bc", BF16)
    bv_bc = _bcast_load(nc, lnA, d["bv"], "bv_bc", BF16)
    pA = tc.alloc_tile_pool(name="pA", bufs=2)
    psA = tc.alloc_tile_pool(name="psA", bufs=2, space="PSUM")

    # x tiles stream on the gpsimd queue; wv (needed in full by the first
    # V matmul group) streams on the scalar queue in parallel.
    xts = []
    for tt in range(TT):
        xt = pA.tile([P, C], BF16, name=f"xt{tt}", tag="xt", bufs=3)
        nc.gpsimd.dma_start(out=xt[:], in_=d["xb"][tt * P:(tt + 1) * P, :])
        xts.append(xt[:])
        if tt < 4:
            nc.scalar.dma_start(wv_t[:, 4 * tt:4 * (tt + 1), :],
                                d["wv"][:, 4 * tt:4 * (tt + 1), :])

    # software-pipelined by one tile: cast(tt)+V(tt) are emitted during
    # LN(tt+1) so the fp8 cast's transpose-wait never blocks the next LN
    # apply in the scalar FIFO.
    xnTts = {}

    def _emit_castv(tt):
        half, lt = divmod(tt, 4)
        nc.scalar.activation(out=xnT8[half][:, :, lt * P:(lt + 1) * P],
                             in_=xnTts.pop(tt)[:], func=ACT.Copy)
        for h in range(H):
            ps_v = psA.tile([P, DH], F32, name="ps_v", tag="psA", bufs=2)
            for kp in range(CP):
                nc.tensor.matmul(ps_v[:],
                                 xnT8[half][:, 2 * kp:2 * kp + 2, lt * P:(lt + 1) * P],
                                 wv_t[:, 2 * kp:2 * kp + 2, h * DH:(h + 1) * DH],
                                 start=(kp == 0), stop=(kp == CP - 1), perf_mode=DR)
            nc.vector.tensor_tensor(vh[:, tt, h * DH:(h + 1) * DH], ps_v[:],
                                    bv_bc[:, h * DH:(h + 1) * DH], OP.add)

    for tt in range(TT):
        xn_t = _ln_tile(nc, pA, xts[tt], l1w_bc, l1b_bc, eps_t, "ln1", tt)
        xnTt = pA.tile([P, CT, P], BF16, name=f"xnTt{tt}", tag="xnTt", bufs=3)
        nc.sync.dma_start_transpose(xnTt[:], xn_t[:])
        xnTts[tt] = xnTt
        if tt > 0:
            _emit_castv(tt - 1)
    _emit_castv(TT - 1)
    psA.release()
    pA.release()
    lnA.release()
    p_wv.release()

    # remaining constants (emitted after stage A so their DMAs don't delay it)
    l2w_bc = _bcast_load(nc, consts, d["l2w"], "l2w_bc", BF16)
    l2b_bc = _bcast_load(nc, consts, d["l2b"], "l2b_bc", BF16)
    bo_bc = _bcast_load(nc, consts, d["bo"], "bo_bc", BF16)
    b2_bc = _bcast_load(nc, consts, d["b2"], "b2_bc", BF16)
    bq_t = consts.tile([P, CT], F32, name="bq_t")
    nc.gpsimd.dma_start(out=bq_t[:], in_=d["bq"])
    bk_t = consts.tile([P, CT], F32, name="bk_t")
    nc.gpsimd.dma_start(out=bk_t[:], in_=d["bk"])
    b1_t = consts.tile([P, FT], F32, name="b1_t")
    nc.gpsimd.dma_start(out=b1_t[:], in_=d["b1"])
    # mask -> additive bias: 0 where visible, -30000 where masked
    p_mb = tc.alloc_tile_pool(name="p_mb", bufs=1)
    mb = p_mb.tile([P, RT, T], BF16, name="mb")
    nc.gpsimd.dma_start(out=mb[:], in_=d["mask"].rearrange("(qo qp) k -> qp qo k", qp=P))
    nc.vector.tensor_scalar(mb[:], mb[:], NEGBIG, -NEGBIG, OP.mult, OP.add)

    # ---------------- Stage B+C: software-pipelined per-head Q/K + attention
    pBC = tc.alloc_tile_pool(name="pBC", bufs=2)
    psBC = tc.alloc_tile_pool(name="psBC", bufs=2, space="PSUM")
    hs = {}

    def emit_qk(h):
        qTh = pBC.tile([P, HT, R], FP8, name=f"qTh{h}", tag="qTh", bufs=2)
        kTh = pBC.tile([P, HT, T], FP8, name=f"kTh{h}", tag="kTh", bufs=2)
        for fl in range(HT):
            fo = h * HT + fl
            wqc = pBC.tile([P, CT, P], FP8, name="wqc", tag="wqc", bufs=2)
            nc.scalar.dma_start(wqc[:], d["wq"][fo])
            wkc = pBC.tile([P, CT, P], FP8, name="wkc", tag="wkc", bufs=2)
            nc.scalar.dma_start(wkc[:], d["wk"][fo])
            ps_q = psBC.tile([P, R], F32, name="ps_q", tag="psB", bufs=2)
            for kp in range(CP):
                nc.tensor.matmul(ps_q[:], wqc[:, 2 * kp:2 * kp + 2, :],
                                 xnT8_lo[:, 2 * kp:2 * kp + 2, :],
                                 start=(kp == 0), stop=(kp == CP - 1), perf_mode=DR)
            nc.scalar.activation(out=qTh[:, fl, :], in_=ps_q[:], func=ACT.Identity,
                                 bias=bq_t[:, fo:fo + 1], scale=1.0)
            for nn in range(2):
                ps_k = psBC.tile([P, 512], F32, name="ps_k", tag="psB", bufs=2)
                for kp in range(CP):
                    nc.tensor.matmul(ps_k[:], wkc[:, 2 * kp:2 * kp + 2, :],
                                     xnT8[nn][:, 2 * kp:2 * kp + 2, :],
                                     start=(kp == 0), stop=(kp == CP - 1), perf_mode=DR)
                nc.scalar.activation(out=kTh[:, fl, nn * 512:(nn + 1) * 512], in_=ps_k[:],
                                     func=ACT.Identity, bias=bk_t[:, fo:fo + 1],
                                     scale=1.0)
        hs[h] = (qTh, kTh)

    def emit_scores(h):
        qTh, kTh = hs[h]
        attT = pBC.tile([P, TT, R], BF16, name=f"attT{h}", tag="attT", bufs=2)
        attT8 = pBC.tile([P, TT, R], FP8, name=f"attT8{h}", tag="attT8", bufs=2)
        for qt in range(RT):
            ps_s = psBC.tile([P, T], F32, name="ps_s", tag="scores", bufs=2)
            for nn in range(2):
                for dp in range(HT // 2):
                    nc.tensor.matmul(
                        ps_s[:, nn * 512:(nn + 1) * 512],
                        qTh[:, 2 * dp:2 * dp + 2, qt * P:(qt + 1) * P],
                        kTh[:, 2 * dp:2 * dp + 2, nn * 512:(nn + 1) * 512],
                        start=(dp == 0), stop=(dp == HT // 2 - 1), perf_mode=DR)
            s_sb = pBC.tile([P, T], F32, name="s_sb", tag="s_sb", bufs=2)
            nc.vector.scalar_tensor_tensor(s_sb[:], ps_s[:], ISQ, mb[:, qt, :],
                                           OP.mult, OP.add)
            # logits are bounded (<= ~15) so exp needs no max-subtraction
            e_sb = pBC.tile([P, T], BF16, name="e_sb", tag="e_sb", bufs=2)
            sums = pBC.tile([P, 1], F32, name="sums", tag="sums", bufs=2)
            nc.scalar.activation(out=e_sb[:], in_=s_sb[:], func=ACT.Exp,
                                 bias=0.0, scale=1.0, accum_out=sums[:])
            recip = pBC.tile([P, 1], F32, name="recip", tag="recip", bufs=2)
            nc.vector.reciprocal(recip[:], sums[:])
            nc.vector.tensor_scalar_mul(e_sb[:], e_sb[:], recip[:])
            nc.sync.dma_start_transpose(attT[:, :, qt * P:(qt + 1) * P], e_sb[:])
        nc.scalar.activation(out=attT8[:], in_=attT[:], func=ACT.Copy)
        hs[h] = hs[h] + (attT8,)

    def emit_av(h):
        _, _, attT8 = hs.pop(h)
        for dt_ in range(HT):
            ps_y = psBC.tile([P, R], F32, name="ps_y", tag="av", bufs=2)
            for kp in range(TT // 2):
                nc.tensor.matmul(
                    ps_y[:],
                    vh[:, 2 * kp:2 * kp + 2, h * DH + dt_ * P:h * DH + (dt_ + 1) * P],
                    attT8[:, 2 * kp:2 * kp + 2, :],
                    start=(kp == 0), stop=(kp == TT // 2 - 1), perf_mode=DR)
            nc.scalar.activation(out=yT[:, h * HT + dt_, :], in_=ps_y[:], func=ACT.Copy)

    emit_qk(0)
    emit_scores(0)
    # preload Wo (4 MB fp8) while attention runs
    for kc in range(4):
        nc.scalar.dma_start(wo_t[:, 4 * kc:4 * (kc + 1), :, :],
                            d["wo"][:, 4 * kc:4 * (kc + 1), :, :])
    for h in range(H):
        if h + 1 < H:
            emit_qk(h + 1)
        emit_av(h)
        if h + 1 < H:
            emit_scores(h + 1)
    psBC.release()
    pBC.release()
    p_mb.release()
    p_vh.release()
    p_xnT.release()

    # ---------------- Stage D: per-qt O-proj + residual + LN2
    # r rows rotate through SBUF (feeding LN2) and bounce to DRAM for stage G.
    p_hT = tc.alloc_tile_pool(name="p_hT", bufs=1)
    hT = p_hT.tile([P, FT, R], BF16, name="hT")
    p_xn2T = tc.alloc_tile_pool(name="p_xn2T", bufs=1)
    xn2T = p_xn2T.tile([P, CT, R], BF16, name="xn2T")
    pD = tc.alloc_tile_pool(name="pD", bufs=2)
    psD = tc.alloc_tile_pool(name="psD", bufs=4, space="PSUM")
    for qt in range(RT):
        xq_t = pD.tile([P, C], F32, name=f"xq{qt}", tag="xq_t", bufs=2)
        nc.gpsimd.dma_start(out=xq_t[:], in_=d["xq"][qt * P:(qt + 1) * P, :])
        r_rot = pD.tile([P, C], F32, name=f"r{qt}", tag="r_rot", bufs=2)
        for fc in range(4):
            ps_o = psD.tile([P, 512], F32, name="ps_o", tag="psD", bufs=4)
            for kp in range(CP):
                nc.tensor.matmul(ps_o[:], yT[:, 2 * kp:2 * kp + 2, qt * P:(qt + 1) * P],
                                 wo_t[:, 2 * kp:2 * kp + 2, fc, :],
                                 start=(kp == 0), stop=(kp == CP - 1), perf_mode=DR)
            r_sl = r_rot[:, fc * 512:(fc + 1) * 512]
            nc.vector.tensor_tensor(r_sl, ps_o[:], bo_bc[:, fc * 512:(fc + 1) * 512],
                                    OP.add)
            nc.vector.tensor_tensor(r_sl, r_sl, xq_t[:, fc * 512:(fc + 1) * 512],
                                    OP.add)
        nc.sync.dma_start(r_d[qt * P:(qt + 1) * P, :], r_rot[:])
        xn2_t = _ln_tile(nc, pD, r_rot[:], l2w_bc, l2b_bc, eps_t, "ln2", qt)
        nc.sync.dma_start_transpose(xn2T[:, :, qt * P:(qt + 1) * P], xn2_t[:])
    psD.release()
    pD.release()

    # ---------------- Stage F: MLP up + gelu -> hT [128, FT, R] bf16
    pF = tc.alloc_tile_pool(name="pF", bufs=2)
    psF = tc.alloc_tile_pool(name="psF", bufs=4, space="PSUM")
    for fp in range(FT // 2):
        w1c = pF.tile([P, CT, 2, P], BF16, name="w1c", tag="w1c", bufs=3)
        nc.scalar.dma_start(w1c[:], d["w1"][fp])
        for fl in range(2):
            fo = 2 * fp + fl
            ps_h = psF.tile([P, R], F32, name="ps_h", tag="psF", bufs=4)
            for ki in range(CT):
                nc.tensor.matmul(ps_h[:], w1c[:, ki, fl, :], xn2T[:, ki, :],
                                 start=(ki == 0), stop=(ki == CT - 1))
            nc.scalar.activation(out=hT[:, fo, :], in_=ps_h[:], func=ACT.Gelu,
                                 bias=b1_t[:, fo:fo + 1], scale=1.0)
    psF.release()
    pF.release()
    p_xn2T.release()

    # ---------------- Stage G: MLP down + residual -> out
    # ones1 has a single 1 in row 0: ones1.T @ b2_bc adds the b2 bias chunk
    # inside the matmul accumulation, so the tail is one DVE add + store.
    ones1 = consts.tile([P, P], BF16, name="ones1")
    nc.vector.memset(ones1[:], 0.0)
    nc.vector.memset(ones1[0:1, :], 1.0)
    pG = tc.alloc_tile_pool(name="pG", bufs=2)
    psG = tc.alloc_tile_pool(name="psG", bufs=8, space="PSUM")
    for fc in range(4):
        ps4 = [psG.tile([P, 512], F32, name=f"ps_g{qt}", tag="psG", bufs=8)
               for qt in range(RT)]
        rgs = []
        for qt in range(RT):
            r_g = pG.tile([P, 512], F32, name="r_g", tag="r_g", bufs=8)
            nc.gpsimd.dma_start(out=r_g[:], in_=r_d[qt * P:(qt + 1) * P,
                                                   fc * 512:(fc + 1) * 512])
            rgs.append(r_g)
        for hb in range(FT // 8):
            w2b = pG.tile([P, 8, 512], BF16, name="w2b", tag="w2b", bufs=3)
            nc.scalar.dma_start(w2b[:], d["w2"][fc, hb])
            for hl in range(8):
                ho = hb * 8 + hl
                for qt in range(RT):
                    nc.tensor.matmul(ps4[qt][:], hT[:, ho, qt * P:(qt + 1) * P],
                                     w2b[:, hl, :], start=(ho == 0), stop=False)
        for qt in range(RT):
            nc.tensor.matmul(ps4[qt][:], ones1[:], b2_bc[:, fc * 512:(fc + 1) * 512],
                             start=False, stop=True)
            o_t = pG.tile([P, 512], F32, name="o_t", tag="o_t", bufs=3)
            nc.vector.tensor_tensor(o_t[:], ps4[qt][:], rgs[qt][:], OP.add)
            nc.sync.dma_start(d["out"][qt * P:(qt + 1) * P, fc * 512:(fc + 1) * 512], o_t[:])
    psG.release()
    pG.release()
    p_hT.release()
    p_wo.release()
    p_yT.release()
    p_rd.release()
    consts.release()


def build_program():
    nc = bacc.Bacc("TRN2", target_bir_lowering=False, debug=False, num_devices=8)
    with tile.TileContext(nc) as tc:
        _body(tc)
    nc.compile()
    return nc


_prog = None


def _get_prog():
    global _prog
    if _prog is None:
        _prog = build_program()
    return _prog


def make_in_maps(x, mask, Wq, bq, Wk, bk, Wv, bv, Wo, bo,
                 ln1_w, ln1_b, ln2_w, ln2_b, W1, b1, W2, b2):
    bf = ml_dtypes.bfloat16
    f8 = ml_dtypes.float8_e4m3
    f32 = np.float32
    cc = np.ascontiguousarray

    def f(a):
        return np.asarray(a, dtype=f32)

    x, mask = np.asarray(x, dtype=f32), np.asarray(mask)
    wq_h = cc(f(Wq).astype(f8).reshape(CT, P, CT, P).transpose(2, 1, 0, 3))
    wk_h = cc(f(Wk).astype(f8).reshape(CT, P, CT, P).transpose(2, 1, 0, 3))
    wv_h = cc(f(Wv).astype(f8).reshape(CT, P, C).transpose(1, 0, 2))
    wo_h = cc(f(Wo).astype(f8).reshape(CT, P, 4, 512).transpose(1, 0, 2, 3))
    w1_h = cc(f(W1).astype(bf).reshape(CT, P, FT // 2, 2, P).transpose(2, 1, 0, 3, 4))
    w2_h = cc(f(W2).astype(bf).reshape(FT // 8, 8, P, 4, 512).transpose(3, 0, 2, 1, 4))
    shared = dict(
        wq=wq_h, wk=wk_h, wv=wv_h, wo=wo_h, w1=w1_h, w2=w2_h,
        bq=cc(f(bq).reshape(CT, P).T), bk=cc(f(bk).reshape(CT, P).T),
        b1=cc(f(b1).reshape(FT, P).T),
        bv=f(bv).astype(bf), bo=f(bo).astype(bf), b2=f(b2).astype(bf),
        l1w=f(ln1_w).astype(bf), l1b=f(ln1_b).astype(bf),
        l2w=f(ln2_w).astype(bf), l2b=f(ln2_b).astype(bf),
    )
    in_maps = []
    for c in range(8):
        b, hh = divmod(c, 2)
        xc = np.roll(x[b], -hh * R, axis=0)
        mk = np.roll(np.asarray(mask[b, hh * R:(hh + 1) * R, :], dtype=f32),
                     -hh * R, axis=1).astype(bf)
        in_maps.append({**shared, "xb": cc(xc.astype(bf)), "xq": cc(xc[:R]),
                        "mask": cc(mk)})
    return in_maps


def kernel(**inputs):
    nc = _get_prog()
    in_maps = make_in_maps(**inputs)
    res = run_bass_kernel_spmd(nc, in_maps, core_ids=list(range(8)))
    out = np.empty((B, T, C), np.float32)
    for c in range(8):
        b, hh = divmod(c, 2)
        out[b, hh * R:(hh + 1) * R, :] = res.results[c]["out"]
    return out


# revision 21
# speedup vs baseline: 1.0545x; 1.0545x over previous
"""Transformer block (LN -> MHA -> residual -> LN -> MLP -> residual) on 8 TRN2
NeuronCores.

Sharding: pure row data-parallelism over (batch, sequence-half). Core c handles
batch b = c//2 and query rows [h*512, (h+1)*512) with h = c%2. Each core
computes K/V projections for its full batch locally (small duplicated work),
which removes every cross-core collective. Host reorders each core's batch rows
"own rows first" so the same SPMD program works on all cores; mask columns are
permuted identically (softmax/attention are permutation-invariant over keys).

v2: overlap restructure + fp8 attention.
  - All attention matmuls (Q/K/V proj, scores, AV, O proj) run in fp8-e4m3
    with perf_mode=DoubleRow (two 128-row contraction tiles per issue),
    ~1.8x the bf16 matmul rate. MLP matmuls stay bf16 (fp8 there blows the
    2e-2 error budget; attention-fp8 costs ~1.3e-2, verified by simulation).
  - LN1 runs per-row-tile; V projections for a token tile are emitted right
    after its transpose, so the PE has work ~10us in instead of idling ~80us.
  - softmax drops the max-subtraction: logits are bounded (|q.k|*isq <~ 15,
    masked lanes are -30000 -> exp==0), so exp never overflows fp32/bf16.
  - LN2 is fused into the O-projection loop per query tile; the attention
    residual r stays in SBUF (no DRAM bounce).
  - output stores stream per (qt, fc) chunk.
fp32 PSUM accumulation everywhere; statistics stay fp32.
"""

import numpy as np
import ml_dtypes

import concourse.bass as bass
import concourse.tile as tile
from concourse import bacc, mybir
from concourse.bass_utils import run_bass_kernel_spmd

BF16 = mybir.dt.bfloat16
F32 = mybir.dt.float32
FP8 = mybir.dt.float8e4
AX = mybir.AxisListType
OP = mybir.AluOpType
ACT = mybir.ActivationFunctionType
DR = mybir.MatmulPerfMode.DoubleRow

P = 128
B, T, C, H = 4, 1024, 2048, 4
DH = C // H                      # 512
F = 4 * C                        # 8192
R = T // 2                       # 512 own query rows per core
RT, TT, CT, FT = R // P, T // P, C // P, F // P   # 4, 8, 16, 64
CP = CT // 2                     # 8 double-row contraction steps over C
HT = DH // P                     # 4 feature tiles per head
EPS = 1e-5
ISQ = 1.0 / float(np.sqrt(DH))
NEGBIG = 30000.0


def _bcast_load(nc, pool, dram_ap, name, dtype):
    """Broadcast a [n] DRAM vector to all 128 partitions -> [128, n]."""
    t = pool.tile([P, dram_ap.shape[0]], dtype, name=name)
    src = bass.AP(
        tensor=dram_ap.tensor, offset=dram_ap.offset, ap=[[0, P]] + list(dram_ap.ap)
    )
    nc.gpsimd.dma_start(out=t[:], in_=src)
    return t


def _ln_tile(nc, pool, x_sl, eps_t, tag, i):
    """Normalize one [128, C] tile -> bf16 (x-mu)*rstd. The LN affine (w,b)
    is folded into the following matmul's weights/biases on the host."""
    stats = pool.tile([P, 4, 6], F32, name=f"{tag}_stats{i}", tag=f"{tag}_stats",
                      bufs=2)
    for sg in range(4):
        nc.vector.bn_stats(out=stats[:, sg, :], in_=x_sl[:, sg * 512:(sg + 1) * 512])
    mv = pool.tile([P, 2], F32, name=f"{tag}_mv{i}", tag=f"{tag}_mv", bufs=2)
    nc.vector.bn_aggr(out=mv[:], in_=stats[:])
    std = pool.tile([P, 1], F32, name=f"{tag}_std{i}", tag=f"{tag}_std", bufs=2)
    nc.scalar.activation(out=std[:], in_=mv[:, 1:2], func=ACT.Sqrt,
                         bias=eps_t[:], scale=1.0)
    rstd = pool.tile([P, 1], F32, name=f"{tag}_rstd{i}", tag=f"{tag}_rstd", bufs=2)
    nc.vector.reciprocal(rstd[:], std[:])
    nmr = pool.tile([P, 1], F32, name=f"{tag}_nmr{i}", tag=f"{tag}_nmr", bufs=2)
    nc.vector.tensor_scalar(nmr[:], mv[:, 0:1], rstd[:], -1.0, OP.mult, OP.mult)
    xh = pool.tile([P, C], BF16, name=f"{tag}_xh{i}", tag=f"{tag}_xh", bufs=2)
    nc.scalar.activation(out=xh[:], in_=x_sl, func=ACT.Identity,
                         bias=nmr[:], scale=rstd[:])
    return xh


def _body(tc):
    nc = tc.nc
    d = {n: nc.dram_tensor(n, s, dt, kind=k).ap() for n, s, dt, k in [
        ("xb", [T, C], BF16, "ExternalInput"),
        ("xq", [R, C], F32, "ExternalInput"),
        ("mask", [R, T], BF16, "ExternalInput"),
        ("wq", [CT, P, CT, P], FP8, "ExternalInput"),
        ("wk", [CT, P, CT, P], FP8, "ExternalInput"),
        ("wv", [P, CT, C], FP8, "ExternalInput"),
        ("wo", [P, CT, 4, 512], FP8, "ExternalInput"),
        ("w1", [FT // 2, P, CT, 2, P], BF16, "ExternalInput"),
        ("w2", [4, FT // 8, P, 8, 512], BF16, "ExternalInput"),
        ("bq", [P, CT], F32, "ExternalInput"),
        ("bk", [P, CT], F32, "ExternalInput"),
        ("b1", [P, FT], F32, "ExternalInput"),
        ("bv", [C], BF16, "ExternalInput"),
        ("bo", [C], BF16, "ExternalInput"),
        ("b2", [C], BF16, "ExternalInput"),
        ("out", [R, C], F32, "ExternalOutput"),
    ]}

    consts = tc.alloc_tile_pool(name="consts", bufs=1)
    eps_t = consts.tile([P, 1], F32, name="eps")
    nc.vector.memset(eps_t[:], EPS)
    p_rd = tc.alloc_tile_pool(name="p_rd", bufs=1, space="DRAM")
    r_d = p_rd.tile([R, C], F32, name="r_d")
    # long-lived SBUF tensors (allocated first: the pool stack is LIFO)
    p_yT = tc.alloc_tile_pool(name="p_yT", bufs=1)
    yT = p_yT.tile([P, CT, R], FP8, name="yT")
    p_wo = tc.alloc_tile_pool(name="p_wo", bufs=1)
    wo_t = p_wo.tile([P, CT, 4, 512], FP8, name="wo_t")
    p_xnT = tc.alloc_tile_pool(name="p_xnT", bufs=1)
    xnT8_lo = p_xnT.tile([P, CT, R], FP8, name="xnT8_lo")
    xnT8_hi = p_xnT.tile([P, CT, R], FP8, name="xnT8_hi")
    xnT8 = [xnT8_lo, xnT8_hi]
    p_vh = tc.alloc_tile_pool(name="p_vh", bufs=1)
    vh = p_vh.tile([P, TT, C], FP8, name="vh")

    # ---------------- Stage A: per-tile LN1 -> transpose -> fp8 cast -> V proj
    p_wv = tc.alloc_tile_pool(name="p_wv", bufs=1)
    wv_t = p_wv.tile([P, CT, C], FP8, name="wv_t")
    lnA = tc.alloc_tile_pool(name="lnA", bufs=1)
    bv_bc = _bcast_load(nc, lnA, d["bv"], "bv_bc", BF16)
    pA = tc.alloc_tile_pool(name="pA", bufs=2)
    psA = tc.alloc_tile_pool(name="psA", bufs=2, space="PSUM")

    # x tiles stream on the gpsimd queue; wv (needed in full by the first
    # V matmul group) streams on the scalar queue in parallel.
    xts = []
    for tt in range(TT):
        xt = pA.tile([P, C], BF16, name=f"xt{tt}", tag="xt", bufs=3)
        nc.sync.dma_start(xt[:], d["xb"][tt * P:(tt + 1) * P, :])
        xts.append(xt[:])
        if tt < 4:
            nc.scalar.dma_start(wv_t[:, 4 * tt:4 * (tt + 1), :],
                                d["wv"][:, 4 * tt:4 * (tt + 1), :])

    # software-pipelined by one tile: cast(tt)+V(tt) are emitted during
    # LN(tt+1) so the fp8 cast's transpose-wait never blocks the next LN
    # apply in the scalar FIFO.
    xnTts = {}

    def _emit_castv(tt):
        half, lt = divmod(tt, 4)
        nc.vector.tensor_copy(xnT8[half][:, :, lt * P:(lt + 1) * P],
                              xnTts.pop(tt)[:])
        for h in range(H):
            ps_v = psA.tile([P, DH], F32, name="ps_v", tag="psA", bufs=2)
            for kp in range(CP):
                nc.tensor.matmul(ps_v[:],
                                 xnT8[half][:, 2 * kp:2 * kp + 2, lt * P:(lt + 1) * P],
                                 wv_t[:, 2 * kp:2 * kp + 2, h * DH:(h + 1) * DH],
                                 start=(kp == 0), stop=(kp == CP - 1), perf_mode=DR)
            nc.vector.tensor_tensor(vh[:, tt, h * DH:(h + 1) * DH], ps_v[:],
                                    bv_bc[:, h * DH:(h + 1) * DH], OP.add)

    for tt in range(TT):
        xn_t = _ln_tile(nc, pA, xts[tt], eps_t, "ln1", tt)
        xnTt = pA.tile([P, CT, P], BF16, name=f"xnTt{tt}", tag="xnTt", bufs=3)
        nc.sync.dma_start_transpose(xnTt[:], xn_t[:])
        xnTts[tt] = xnTt
        if tt > 0:
            _emit_castv(tt - 1)
    _emit_castv(TT - 1)
    psA.release()
    pA.release()
    lnA.release()
    p_wv.release()

    # remaining constants (emitted after stage A so their DMAs don't delay it)
    bo_bc = _bcast_load(nc, consts, d["bo"], "bo_bc", BF16)
    b2_bc = _bcast_load(nc, consts, d["b2"], "b2_bc", BF16)
    bq_t = consts.tile([P, CT], F32, name="bq_t")
    nc.gpsimd.dma_start(out=bq_t[:], in_=d["bq"])
    bk_t = consts.tile([P, CT], F32, name="bk_t")
    nc.gpsimd.dma_start(out=bk_t[:], in_=d["bk"])
    b1_t = consts.tile([P, FT], F32, name="b1_t")
    nc.gpsimd.dma_start(out=b1_t[:], in_=d["b1"])
    # mask -> additive bias: 0 where visible, -30000 where masked
    p_mb = tc.alloc_tile_pool(name="p_mb", bufs=1)
    mb = p_mb.tile([P, RT, T], BF16, name="mb")
    nc.gpsimd.dma_start(out=mb[:], in_=d["mask"].rearrange("(qo qp) k -> qp qo k", qp=P))
    nc.vector.tensor_scalar(mb[:], mb[:], NEGBIG, -NEGBIG, OP.mult, OP.add)

    # ---------------- Stage B+C: software-pipelined per-head Q/K + attention
    pBC = tc.alloc_tile_pool(name="pBC", bufs=2)
    psBC = tc.alloc_tile_pool(name="psBC", bufs=2, space="PSUM")
    hs = {}

    def emit_qk(h):
        qTh = pBC.tile([P, HT, R], FP8, name=f"qTh{h}", tag="qTh", bufs=2)
        kTh = pBC.tile([P, HT, T], FP8, name=f"kTh{h}", tag="kTh", bufs=2)
        wqcs, wkcs = [], []
        for fl in range(HT):
            fo = h * HT + fl
            wqc = pBC.tile([P, CT, P], FP8, name="wqc", tag="wqc", bufs=4)
            nc.sync.dma_start(wqc[:], d["wq"][fo])
            wqcs.append(wqc)
            wkc = pBC.tile([P, CT, P], FP8, name="wkc", tag="wkc", bufs=4)
            nc.sync.dma_start(wkc[:], d["wk"][fo])
            wkcs.append(wkc)
        for fl in range(HT):
            fo = h * HT + fl
            wqc, wkc = wqcs[fl], wkcs[fl]
            ps_q = psBC.tile([P, R], F32, name="ps_q", tag="psB", bufs=2)
            for kp in range(CP):
                nc.tensor.matmul(ps_q[:], wqc[:, 2 * kp:2 * kp + 2, :],
                                 xnT8_lo[:, 2 * kp:2 * kp + 2, :],
                                 start=(kp == 0), stop=(kp == CP - 1), perf_mode=DR)
            nc.scalar.activation(out=qTh[:, fl, :], in_=ps_q[:], func=ACT.Identity,
                                 bias=bq_t[:, fo:fo + 1], scale=1.0)
            for nn in range(2):
                ps_k = psBC.tile([P, 512], F32, name="ps_k", tag="psB", bufs=2)
                for kp in range(CP):
                    nc.tensor.matmul(ps_k[:], wkc[:, 2 * kp:2 * kp + 2, :],
                                     xnT8[nn][:, 2 * kp:2 * kp + 2, :],
                                     start=(kp == 0), stop=(kp == CP - 1), perf_mode=DR)
                nc.scalar.activation(out=kTh[:, fl, nn * 512:(nn + 1) * 512], in_=ps_k[:],
                                     func=ACT.Identity, bias=bk_t[:, fo:fo + 1],
                                     scale=1.0)
        hs[h] = (qTh, kTh)

    def emit_scores(h):
        qTh, kTh = hs[h]
        attT = pBC.tile([P, TT, R], BF16, name=f"attT{h}", tag="attT", bufs=2)
        attT8 = pBC.tile([P, TT, R], FP8, name=f"attT8{h}", tag="attT8", bufs=2)
        for qt in range(RT):
            ps_s = psBC.tile([P, T], F32, name="ps_s", tag="scores", bufs=2)
            for nn in range(2):
                for dp in range(HT // 2):
                    nc.tensor.matmul(
                        ps_s[:, nn * 512:(nn + 1) * 512],
                        qTh[:, 2 * dp:2 * dp + 2, qt * P:(qt + 1) * P],
                        kTh[:, 2 * dp:2 * dp + 2, nn * 512:(nn + 1) * 512],
                        start=(dp == 0), stop=(dp == HT // 2 - 1), perf_mode=DR)
            s_sb = pBC.tile([P, T], F32, name="s_sb", tag="s_sb", bufs=2)
            nc.vector.scalar_tensor_tensor(s_sb[:], ps_s[:], ISQ, mb[:, qt, :],
                                           OP.mult, OP.add)
            # logits are bounded (<= ~15) so exp needs no max-subtraction
            e_sb = pBC.tile([P, T], BF16, name="e_sb", tag="e_sb", bufs=2)
            sums = pBC.tile([P, 1], F32, name="sums", tag="sums", bufs=2)
            nc.scalar.activation(out=e_sb[:], in_=s_sb[:], func=ACT.Exp,
                                 bias=0.0, scale=1.0, accum_out=sums[:])
            recip = pBC.tile([P, 1], F32, name="recip", tag="recip", bufs=2)
            nc.vector.reciprocal(recip[:], sums[:])
            nc.vector.tensor_scalar_mul(e_sb[:], e_sb[:], recip[:])
            nc.sync.dma_start_transpose(attT[:, :, qt * P:(qt + 1) * P], e_sb[:])
        nc.vector.tensor_copy(attT8[:], attT[:])
        hs[h] = hs[h] + (attT8,)

    def emit_av(h):
        _, _, attT8 = hs.pop(h)
        for dt_ in range(HT):
            ps_y = psBC.tile([P, R], F32, name="ps_y", tag="av", bufs=2)
            for kp in range(TT // 2):
                nc.tensor.matmul(
                    ps_y[:],
                    vh[:, 2 * kp:2 * kp + 2, h * DH + dt_ * P:h * DH + (dt_ + 1) * P],
                    attT8[:, 2 * kp:2 * kp + 2, :],
                    start=(kp == 0), stop=(kp == TT // 2 - 1), perf_mode=DR)
            nc.scalar.activation(out=yT[:, h * HT + dt_, :], in_=ps_y[:], func=ACT.Copy)

    emit_qk(0)
    emit_scores(0)
    # preload Wo (4 MB fp8) while attention runs
    for kc in range(4):
        nc.scalar.dma_start(wo_t[:, 4 * kc:4 * (kc + 1), :, :],
                            d["wo"][:, 4 * kc:4 * (kc + 1), :, :])
    for h in range(H):
        if h + 1 < H:
            emit_qk(h + 1)
        emit_av(h)
        if h + 1 < H:
            emit_scores(h + 1)
    psBC.release()
    pBC.release()
    p_mb.release()
    p_vh.release()
    p_xnT.release()

    # ones1 has a single 1 in row 0: ones1.T @ bias_bc adds a bias chunk
    # inside the matmul accumulation (used in stages D and G).
    ones1 = consts.tile([P, P], BF16, name="ones1")
    nc.vector.memset(ones1[:], 0.0)
    nc.vector.memset(ones1[0:1, :], 1.0)

    # ---------------- Stage D: per-qt O-proj + residual + LN2
    # r rows rotate through SBUF (feeding LN2) and bounce to DRAM for stage G.
    p_hT = tc.alloc_tile_pool(name="p_hT", bufs=1)
    hT = p_hT.tile([P, FT, R], BF16, name="hT")
    p_xn2T = tc.alloc_tile_pool(name="p_xn2T", bufs=1)
    xn2T = p_xn2T.tile([P, CT, R], BF16, name="xn2T")
    pD = tc.alloc_tile_pool(name="pD", bufs=2)
    psD = tc.alloc_tile_pool(name="psD", bufs=4, space="PSUM")
    for qt in range(RT):
        xq_t = pD.tile([P, C], F32, name=f"xq{qt}", tag="xq_t", bufs=2)
        nc.sync.dma_start(xq_t[:], d["xq"][qt * P:(qt + 1) * P, :])
        r_rot = pD.tile([P, C], F32, name=f"r{qt}", tag="r_rot", bufs=2)
        for fc in range(4):
            ps_o = psD.tile([P, 512], F32, name="ps_o", tag="psD", bufs=4)
            for kp in range(CP):
                nc.tensor.matmul(ps_o[:], yT[:, 2 * kp:2 * kp + 2, qt * P:(qt + 1) * P],
                                 wo_t[:, 2 * kp:2 * kp + 2, fc, :],
                                 start=(kp == 0), stop=False, perf_mode=DR)
            nc.tensor.matmul(ps_o[:], ones1[:], bo_bc[:, fc * 512:(fc + 1) * 512],
                             start=False, stop=True)
            r_sl = r_rot[:, fc * 512:(fc + 1) * 512]
            nc.vector.tensor_tensor(r_sl, ps_o[:], xq_t[:, fc * 512:(fc + 1) * 512],
                                    OP.add)
        nc.sync.dma_start(r_d[qt * P:(qt + 1) * P, :], r_rot[:])
        xn2_t = _ln_tile(nc, pD, r_rot[:], eps_t, "ln2", qt)
        nc.sync.dma_start_transpose(xn2T[:, :, qt * P:(qt + 1) * P], xn2_t[:])
    psD.release()
    pD.release()

    # ---------------- Stage F: MLP up + gelu -> hT [128, FT, R] bf16
    pF = tc.alloc_tile_pool(name="pF", bufs=2)
    psF = tc.alloc_tile_pool(name="psF", bufs=4, space="PSUM")
    for fp in range(FT // 2):
        w1c = pF.tile([P, CT, 2, P], BF16, name="w1c", tag="w1c", bufs=3)
        nc.scalar.dma_start(w1c[:], d["w1"][fp])
        for fl in range(2):
            fo = 2 * fp + fl
            ps_h = psF.tile([P, R], F32, name="ps_h", tag="psF", bufs=4)
            for ki in range(CT):
                nc.tensor.matmul(ps_h[:], w1c[:, ki, fl, :], xn2T[:, ki, :],
                                 start=(ki == 0), stop=(ki == CT - 1))
            nc.scalar.activation(out=hT[:, fo, :], in_=ps_h[:], func=ACT.Gelu,
                                 bias=b1_t[:, fo:fo + 1], scale=1.0)
    psF.release()
    pF.release()
    p_xn2T.release()

    # ---------------- Stage G: MLP down + residual -> out
    pG = tc.alloc_tile_pool(name="pG", bufs=2)
    psG = tc.alloc_tile_pool(name="psG", bufs=8, space="PSUM")
    for fc in range(4):
        ps4 = [psG.tile([P, 512], F32, name=f"ps_g{qt}", tag="psG", bufs=8)
               for qt in range(RT)]
        rgs = []
        for qt in range(RT):
            r_g = pG.tile([P, 512], F32, name="r_g", tag="r_g", bufs=8)
            nc.sync.dma_start(r_g[:], r_d[qt * P:(qt + 1) * P,
                                          fc * 512:(fc + 1) * 512])
            rgs.append(r_g)
        for hb in range(FT // 8):
            w2b = pG.tile([P, 8, 512], BF16, name="w2b", tag="w2b", bufs=3)
            nc.scalar.dma_start(w2b[:], d["w2"][fc, hb])
            for hl in range(8):
                ho = hb * 8 + hl
                for qt in range(RT):
                    nc.tensor.matmul(ps4[qt][:], hT[:, ho, qt * P:(qt + 1) * P],
                                     w2b[:, hl, :], start=(ho == 0), stop=False)
        for qt in range(RT):
            nc.tensor.matmul(ps4[qt][:], ones1[:], b2_bc[:, fc * 512:(fc + 1) * 512],
                             start=False, stop=True)
            o_t = pG.tile([P, 512], F32, name="o_t", tag="o_t", bufs=3)
            nc.vector.tensor_tensor(o_t[:], ps4[qt][:], rgs[qt][:], OP.add)
            nc.sync.dma_start(d["out"][qt * P:(qt + 1) * P, fc * 512:(fc + 1) * 512], o_t[:])
    psG.release()
    pG.release()
    p_hT.release()
    p_wo.release()
    p_yT.release()
    p_rd.release()
    consts.release()


def build_program():
    nc = bacc.Bacc("TRN2", target_bir_lowering=False, debug=False, num_devices=8)
    with tile.TileContext(nc) as tc:
        _body(tc)
    nc.compile()
    return nc


_prog = None


def _get_prog():
    global _prog
    if _prog is None:
        _prog = build_program()
    return _prog


def make_in_maps(x, mask, Wq, bq, Wk, bk, Wv, bv, Wo, bo,
                 ln1_w, ln1_b, ln2_w, ln2_b, W1, b1, W2, b2):
    bf = ml_dtypes.bfloat16
    f8 = ml_dtypes.float8_e4m3
    f32 = np.float32
    cc = np.ascontiguousarray

    def f(a):
        return np.asarray(a, dtype=f32)

    x, mask = np.asarray(x, dtype=f32), np.asarray(mask)
    # fold the LN affines into the consuming matmuls: for y = ln(x)@W + b with
    # ln(x) = z*w + b_ln (z the normalized input), y = z@(w[:,None]*W) + (b_ln@W + b)
    w1l, b1l = f(ln1_w)[:, None], f(ln1_b)
    w2l, b2l = f(ln2_w)[:, None], f(ln2_b)
    Wq_, Wk_, Wv_, W1_ = w1l * f(Wq), w1l * f(Wk), w1l * f(Wv), w2l * f(W1)
    bq_, bk_ = f(bq) + b1l @ f(Wq), f(bk) + b1l @ f(Wk)
    bv_, b1_ = f(bv) + b1l @ f(Wv), f(b1) + b2l @ f(W1)
    wq_h = cc(Wq_.astype(f8).reshape(CT, P, CT, P).transpose(2, 1, 0, 3))
    wk_h = cc(Wk_.astype(f8).reshape(CT, P, CT, P).transpose(2, 1, 0, 3))
    wv_h = cc(Wv_.astype(f8).reshape(CT, P, C).transpose(1, 0, 2))
    wo_h = cc(f(Wo).astype(f8).reshape(CT, P, 4, 512).transpose(1, 0, 2, 3))
    w1_h = cc(W1_.astype(bf).reshape(CT, P, FT // 2, 2, P).transpose(2, 1, 0, 3, 4))
    w2_h = cc(f(W2).astype(bf).reshape(FT // 8, 8, P, 4, 512).transpose(3, 0, 2, 1, 4))
    shared = dict(
        wq=wq_h, wk=wk_h, wv=wv_h, wo=wo_h, w1=w1_h, w2=w2_h,
        bq=cc(bq_.reshape(CT, P).T), bk=cc(bk_.reshape(CT, P).T),
        b1=cc(b1_.reshape(FT, P).T),
        bv=bv_.astype(bf), bo=f(bo).astype(bf), b2=f(b2).astype(bf),
    )
    in_maps = []
    for c in range(8):
        b, hh = divmod(c, 2)
        xc = np.roll(x[b], -hh * R, axis=0)
        mk = np.roll(np.asarray(mask[b, hh * R:(hh + 1) * R, :], dtype=f32),
                     -hh * R, axis=1).astype(bf)
        in_maps.append({**shared, "xb": cc(xc.astype(bf)), "xq": cc(xc[:R]),
                        "mask": cc(mk)})
    return in_maps


def kernel(**inputs):
    nc = _get_prog()
    in_maps = make_in_maps(**inputs)
    res = run_bass_kernel_spmd(nc, in_maps, core_ids=list(range(8)))
    out = np.empty((B, T, C), np.float32)
    for c in range(8):
        b, hh = divmod(c, 2)
        out[b, hh * R:(hh + 1) * R, :] = res.results[c]["out"]
    return out


# revision 23
# speedup vs baseline: 1.0553x; 1.0007x over previous
"""Transformer block (LN -> MHA -> residual -> LN -> MLP -> residual) on 8 TRN2
NeuronCores.

Sharding: pure row data-parallelism over (batch, sequence-half). Core c handles
batch b = c//2 and query rows [h*512, (h+1)*512) with h = c%2. Each core
computes K/V projections for its full batch locally (small duplicated work),
which removes every cross-core collective. Host reorders each core's batch rows
"own rows first" so the same SPMD program works on all cores; mask columns are
permuted identically (softmax/attention are permutation-invariant over keys).

v2: overlap restructure + fp8 attention.
  - All attention matmuls (Q/K/V proj, scores, AV, O proj) run in fp8-e4m3
    with perf_mode=DoubleRow (two 128-row contraction tiles per issue),
    ~1.8x the bf16 matmul rate. MLP matmuls stay bf16 (fp8 there blows the
    2e-2 error budget; attention-fp8 costs ~1.3e-2, verified by simulation).
  - LN1 runs per-row-tile; V projections for a token tile are emitted right
    after its transpose, so the PE has work ~10us in instead of idling ~80us.
  - softmax drops the max-subtraction: logits are bounded (|q.k|*isq <~ 15,
    masked lanes are -30000 -> exp==0), so exp never overflows fp32/bf16.
  - LN2 is fused into the O-projection loop per query tile; the attention
    residual r stays in SBUF (no DRAM bounce).
  - output stores stream per (qt, fc) chunk.
fp32 PSUM accumulation everywhere; statistics stay fp32.
"""

import numpy as np
import ml_dtypes

import concourse.bass as bass
import concourse.tile as tile
from concourse import bacc, mybir
from concourse.bass_utils import run_bass_kernel_spmd

BF16 = mybir.dt.bfloat16
F32 = mybir.dt.float32
FP8 = mybir.dt.float8e4
AX = mybir.AxisListType
OP = mybir.AluOpType
ACT = mybir.ActivationFunctionType
DR = mybir.MatmulPerfMode.DoubleRow

P = 128
B, T, C, H = 4, 1024, 2048, 4
DH = C // H                      # 512
F = 4 * C                        # 8192
R = T // 2                       # 512 own query rows per core
RT, TT, CT, FT = R // P, T // P, C // P, F // P   # 4, 8, 16, 64
CP = CT // 2                     # 8 double-row contraction steps over C
HT = DH // P                     # 4 feature tiles per head
EPS = 1e-5
ISQ = 1.0 / float(np.sqrt(DH))
NEGBIG = 30000.0


def _bcast_load(nc, pool, dram_ap, name, dtype):
    """Broadcast a [n] DRAM vector to all 128 partitions -> [128, n]."""
    t = pool.tile([P, dram_ap.shape[0]], dtype, name=name)
    src = bass.AP(
        tensor=dram_ap.tensor, offset=dram_ap.offset, ap=[[0, P]] + list(dram_ap.ap)
    )
    nc.gpsimd.dma_start(out=t[:], in_=src)
    return t


def _ln_tile(nc, pool, x_sl, eps_t, tag, i):
    """Normalize one [128, C] tile -> bf16 (x-mu)*rstd. The LN affine (w,b)
    is folded into the following matmul's weights/biases on the host."""
    stats = pool.tile([P, 4, 6], F32, name=f"{tag}_stats{i}", tag=f"{tag}_stats",
                      bufs=2)
    for sg in range(4):
        nc.vector.bn_stats(out=stats[:, sg, :], in_=x_sl[:, sg * 512:(sg + 1) * 512])
    mv = pool.tile([P, 2], F32, name=f"{tag}_mv{i}", tag=f"{tag}_mv", bufs=2)
    nc.vector.bn_aggr(out=mv[:], in_=stats[:])
    std = pool.tile([P, 1], F32, name=f"{tag}_std{i}", tag=f"{tag}_std", bufs=2)
    nc.scalar.activation(out=std[:], in_=mv[:, 1:2], func=ACT.Sqrt,
                         bias=eps_t[:], scale=1.0)
    rstd = pool.tile([P, 1], F32, name=f"{tag}_rstd{i}", tag=f"{tag}_rstd", bufs=2)
    nc.vector.reciprocal(rstd[:], std[:])
    nmr = pool.tile([P, 1], F32, name=f"{tag}_nmr{i}", tag=f"{tag}_nmr", bufs=2)
    nc.vector.tensor_scalar(nmr[:], mv[:, 0:1], rstd[:], -1.0, OP.mult, OP.mult)
    xh = pool.tile([P, C], BF16, name=f"{tag}_xh{i}", tag=f"{tag}_xh", bufs=2)
    nc.scalar.activation(out=xh[:], in_=x_sl, func=ACT.Identity,
                         bias=nmr[:], scale=rstd[:])
    return xh


def _body(tc):
    nc = tc.nc
    d = {n: nc.dram_tensor(n, s, dt, kind=k).ap() for n, s, dt, k in [
        ("xb", [T, C], BF16, "ExternalInput"),
        ("xq", [R, C], F32, "ExternalInput"),
        ("mask", [R, T], BF16, "ExternalInput"),
        ("wq", [CT, P, CT, P], FP8, "ExternalInput"),
        ("wk", [CT, P, CT, P], FP8, "ExternalInput"),
        ("wv", [P, CT, C], FP8, "ExternalInput"),
        ("wo", [P, CT, 4, 512], FP8, "ExternalInput"),
        ("w1", [FT // 2, P, CT, 2, P], BF16, "ExternalInput"),
        ("w2", [4, FT // 8, P, 8, 512], BF16, "ExternalInput"),
        ("bq", [P, CT], F32, "ExternalInput"),
        ("bk", [P, CT], F32, "ExternalInput"),
        ("b1", [P, FT], F32, "ExternalInput"),
        ("bv", [C], BF16, "ExternalInput"),
        ("bo", [C], BF16, "ExternalInput"),
        ("b2", [C], BF16, "ExternalInput"),
        ("out", [R, C], F32, "ExternalOutput"),
    ]}

    consts = tc.alloc_tile_pool(name="consts", bufs=1)
    eps_t = consts.tile([P, 1], F32, name="eps")
    nc.vector.memset(eps_t[:], EPS)
    p_rd = tc.alloc_tile_pool(name="p_rd", bufs=1, space="DRAM")
    r_d = p_rd.tile([R, C], F32, name="r_d")
    # long-lived SBUF tensors (allocated first: the pool stack is LIFO)
    p_yT = tc.alloc_tile_pool(name="p_yT", bufs=1)
    yT = p_yT.tile([P, CT, R], FP8, name="yT")
    p_wo = tc.alloc_tile_pool(name="p_wo", bufs=1)
    wo_t = p_wo.tile([P, CT, 4, 512], FP8, name="wo_t")
    p_xnT = tc.alloc_tile_pool(name="p_xnT", bufs=1)
    xnT8_lo = p_xnT.tile([P, CT, R], FP8, name="xnT8_lo")
    xnT8_hi = p_xnT.tile([P, CT, R], FP8, name="xnT8_hi")
    xnT8 = [xnT8_lo, xnT8_hi]
    p_vh = tc.alloc_tile_pool(name="p_vh", bufs=1)
    vh = p_vh.tile([P, TT, C], FP8, name="vh")

    # ---------------- Stage A: per-tile LN1 -> transpose -> fp8 cast -> V proj
    p_wv = tc.alloc_tile_pool(name="p_wv", bufs=1)
    wv_t = p_wv.tile([P, CT, C], FP8, name="wv_t")
    lnA = tc.alloc_tile_pool(name="lnA", bufs=1)
    bv_bc = _bcast_load(nc, lnA, d["bv"], "bv_bc", BF16)
    pA = tc.alloc_tile_pool(name="pA", bufs=2)
    psA = tc.alloc_tile_pool(name="psA", bufs=2, space="PSUM")

    # x tiles stream on the gpsimd queue; wv (needed in full by the first
    # V matmul group) streams on the scalar queue in parallel.
    xts = []
    for tt in range(TT):
        xt = pA.tile([P, C], BF16, name=f"xt{tt}", tag="xt", bufs=3)
        nc.sync.dma_start(xt[:], d["xb"][tt * P:(tt + 1) * P, :])
        xts.append(xt[:])
    for kc in range(2):
        nc.scalar.dma_start(wv_t[:, 4 * kc:4 * (kc + 1), :],
                            d["wv"][:, 4 * kc:4 * (kc + 1), :])

    # software-pipelined by one tile: cast(tt)+V(tt) are emitted during
    # LN(tt+1) so the fp8 cast's transpose-wait never blocks the next LN
    # apply in the scalar FIFO.
    xnTts = {}

    def _emit_castv(tt):
        half, lt = divmod(tt, 4)
        nc.vector.tensor_copy(xnT8[half][:, :, lt * P:(lt + 1) * P],
                              xnTts.pop(tt)[:])
        for h in range(H):
            ps_v = psA.tile([P, DH], F32, name="ps_v", tag="psA", bufs=2)
            for kp in range(CP):
                nc.tensor.matmul(ps_v[:],
                                 xnT8[half][:, 2 * kp:2 * kp + 2, lt * P:(lt + 1) * P],
                                 wv_t[:, 2 * kp:2 * kp + 2, h * DH:(h + 1) * DH],
                                 start=(kp == 0), stop=(kp == CP - 1), perf_mode=DR)
            nc.vector.tensor_tensor(vh[:, tt, h * DH:(h + 1) * DH], ps_v[:],
                                    bv_bc[:, h * DH:(h + 1) * DH], OP.add)

    for tt in range(TT):
        xn_t = _ln_tile(nc, pA, xts[tt], eps_t, "ln1", tt)
        xnTt = pA.tile([P, CT, P], BF16, name=f"xnTt{tt}", tag="xnTt", bufs=3)
        nc.scalar.dma_start_transpose(xnTt[:], xn_t[:])
        xnTts[tt] = xnTt
        if tt < 2:  # interleave the rest of wv behind the first transposes
            nc.scalar.dma_start(wv_t[:, 4 * (tt + 2):4 * (tt + 3), :],
                                d["wv"][:, 4 * (tt + 2):4 * (tt + 3), :])
        if tt > 0:
            _emit_castv(tt - 1)
    _emit_castv(TT - 1)
    psA.release()
    pA.release()
    lnA.release()
    p_wv.release()

    # remaining constants (emitted after stage A so their DMAs don't delay it)
    bo_bc = _bcast_load(nc, consts, d["bo"], "bo_bc", BF16)
    b2_bc = _bcast_load(nc, consts, d["b2"], "b2_bc", BF16)
    bq_t = consts.tile([P, CT], F32, name="bq_t")
    nc.gpsimd.dma_start(out=bq_t[:], in_=d["bq"])
    bk_t = consts.tile([P, CT], F32, name="bk_t")
    nc.gpsimd.dma_start(out=bk_t[:], in_=d["bk"])
    b1_t = consts.tile([P, FT], F32, name="b1_t")
    nc.gpsimd.dma_start(out=b1_t[:], in_=d["b1"])
    # mask -> additive bias: 0 where visible, -30000 where masked
    p_mb = tc.alloc_tile_pool(name="p_mb", bufs=1)
    mb = p_mb.tile([P, RT, T], BF16, name="mb")
    nc.gpsimd.dma_start(out=mb[:], in_=d["mask"].rearrange("(qo qp) k -> qp qo k", qp=P))
    nc.vector.tensor_scalar(mb[:], mb[:], NEGBIG, -NEGBIG, OP.mult, OP.add)

    # ---------------- Stage B+C: software-pipelined per-head Q/K + attention
    pBC = tc.alloc_tile_pool(name="pBC", bufs=2)
    psBC = tc.alloc_tile_pool(name="psBC", bufs=2, space="PSUM")
    hs = {}

    def emit_qk(h):
        qTh = pBC.tile([P, HT, R], FP8, name=f"qTh{h}", tag="qTh", bufs=2)
        kTh = pBC.tile([P, HT, T], FP8, name=f"kTh{h}", tag="kTh", bufs=2)
        eng = nc.sync if h == 0 else nc.scalar
        wqcs, wkcs = [], []
        for fl in range(HT):
            fo = h * HT + fl
            wqc = pBC.tile([P, CT, P], FP8, name="wqc", tag="wqc", bufs=4)
            eng.dma_start(wqc[:], d["wq"][fo])
            wqcs.append(wqc)
            wkc = pBC.tile([P, CT, P], FP8, name="wkc", tag="wkc", bufs=4)
            eng.dma_start(wkc[:], d["wk"][fo])
            wkcs.append(wkc)
        for fl in range(HT):
            fo = h * HT + fl
            wqc, wkc = wqcs[fl], wkcs[fl]
            ps_q = psBC.tile([P, R], F32, name="ps_q", tag="psB", bufs=2)
            for kp in range(CP):
                nc.tensor.matmul(ps_q[:], wqc[:, 2 * kp:2 * kp + 2, :],
                                 xnT8_lo[:, 2 * kp:2 * kp + 2, :],
                                 start=(kp == 0), stop=(kp == CP - 1), perf_mode=DR)
            nc.scalar.activation(out=qTh[:, fl, :], in_=ps_q[:], func=ACT.Identity,
                                 bias=bq_t[:, fo:fo + 1], scale=1.0)
            for nn in range(2):
                ps_k = psBC.tile([P, 512], F32, name="ps_k", tag="psB", bufs=2)
                for kp in range(CP):
                    nc.tensor.matmul(ps_k[:], wkc[:, 2 * kp:2 * kp + 2, :],
                                     xnT8[nn][:, 2 * kp:2 * kp + 2, :],
                                     start=(kp == 0), stop=(kp == CP - 1), perf_mode=DR)
                nc.scalar.activation(out=kTh[:, fl, nn * 512:(nn + 1) * 512], in_=ps_k[:],
                                     func=ACT.Identity, bias=bk_t[:, fo:fo + 1],
                                     scale=1.0)
        hs[h] = (qTh, kTh)

    def emit_scores(h):
        qTh, kTh = hs[h]
        attT = pBC.tile([P, TT, R], BF16, name=f"attT{h}", tag="attT", bufs=2)
        attT8 = pBC.tile([P, TT, R], FP8, name=f"attT8{h}", tag="attT8", bufs=2)
        for qt in range(RT):
            ps_s = psBC.tile([P, T], F32, name="ps_s", tag="scores", bufs=2)
            for nn in range(2):
                for dp in range(HT // 2):
                    nc.tensor.matmul(
                        ps_s[:, nn * 512:(nn + 1) * 512],
                        qTh[:, 2 * dp:2 * dp + 2, qt * P:(qt + 1) * P],
                        kTh[:, 2 * dp:2 * dp + 2, nn * 512:(nn + 1) * 512],
                        start=(dp == 0), stop=(dp == HT // 2 - 1), perf_mode=DR)
            s_sb = pBC.tile([P, T], F32, name="s_sb", tag="s_sb", bufs=2)
            nc.vector.scalar_tensor_tensor(s_sb[:], ps_s[:], ISQ, mb[:, qt, :],
                                           OP.mult, OP.add)
            # logits are bounded (<= ~15) so exp needs no max-subtraction
            e_sb = pBC.tile([P, T], BF16, name="e_sb", tag="e_sb", bufs=2)
            sums = pBC.tile([P, 1], F32, name="sums", tag="sums", bufs=2)
            nc.scalar.activation(out=e_sb[:], in_=s_sb[:], func=ACT.Exp,
                                 bias=0.0, scale=1.0, accum_out=sums[:])
            recip = pBC.tile([P, 1], F32, name="recip", tag="recip", bufs=2)
            nc.vector.reciprocal(recip[:], sums[:])
            nc.vector.tensor_scalar_mul(e_sb[:], e_sb[:], recip[:])
            nc.sync.dma_start_transpose(attT[:, :, qt * P:(qt + 1) * P], e_sb[:])
        if h == H - 1:  # per-qt cast so the split AV can start immediately
            for qt in range(RT):
                nc.vector.tensor_copy(attT8[:, :, qt * P:(qt + 1) * P],
                                      attT[:, :, qt * P:(qt + 1) * P])
        else:
            nc.vector.tensor_copy(attT8[:], attT[:])
        hs[h] = hs[h] + (attT8,)

    def emit_av(h):
        _, _, attT8 = hs.pop(h)
        nq = RT if h == H - 1 else 1   # last head: split over qt chunks so AV
        nw = R // nq                   # overlaps the tail softmax chain
        for dt_ in range(HT):
            ps_y = psBC.tile([P, R], F32, name="ps_y", tag="av", bufs=2)
            for qc in range(nq):
                for kp in range(TT // 2):
                    nc.tensor.matmul(
                        ps_y[:, qc * nw:(qc + 1) * nw],
                        vh[:, 2 * kp:2 * kp + 2, h * DH + dt_ * P:h * DH + (dt_ + 1) * P],
                        attT8[:, 2 * kp:2 * kp + 2, qc * nw:(qc + 1) * nw],
                        start=(kp == 0), stop=(kp == TT // 2 - 1), perf_mode=DR)
            nc.scalar.activation(out=yT[:, h * HT + dt_, :], in_=ps_y[:], func=ACT.Copy)

    emit_qk(0)
    emit_scores(0)
    # preload Wo (4 MB fp8) while attention runs
    for kc in range(4):
        nc.scalar.dma_start(wo_t[:, 4 * kc:4 * (kc + 1), :, :],
                            d["wo"][:, 4 * kc:4 * (kc + 1), :, :])
    for h in range(H):
        if h + 1 < H:
            emit_qk(h + 1)
        emit_av(h)
        if h + 1 < H:
            emit_scores(h + 1)
    psBC.release()
    pBC.release()
    p_mb.release()
    p_vh.release()
    p_xnT.release()

    # ones1 has a single 1 in row 0: ones1.T @ bias_bc adds a bias chunk
    # inside the matmul accumulation (used in stages D and G).
    ones1 = consts.tile([P, P], BF16, name="ones1")
    nc.vector.memset(ones1[:], 0.0)
    nc.vector.memset(ones1[0:1, :], 1.0)

    # ---------------- Stage D: per-qt O-proj + residual + LN2
    # r rows rotate through SBUF (feeding LN2) and bounce to DRAM for stage G.
    # One shared PSUM pool (psX) serves D, F and G so no stage-boundary
    # write-after-read stall on fresh PSUM banks.
    p_hT = tc.alloc_tile_pool(name="p_hT", bufs=1)
    hT = p_hT.tile([P, FT, R], BF16, name="hT")
    psX = tc.alloc_tile_pool(name="psX", bufs=8, space="PSUM")
    p_xn2T = tc.alloc_tile_pool(name="p_xn2T", bufs=1)
    xn2T = p_xn2T.tile([P, CT, R], BF16, name="xn2T")
    pD = tc.alloc_tile_pool(name="pD", bufs=2)
    xqs = []
    for qt in range(RT):
        xq_t = pD.tile([P, C], F32, name=f"xq{qt}", tag="xq_t", bufs=4)
        nc.sync.dma_start(xq_t[:], d["xq"][qt * P:(qt + 1) * P, :])
        xqs.append(xq_t)
    for qt in range(RT):
        xq_t = xqs[qt]
        r_rot = pD.tile([P, C], F32, name=f"r{qt}", tag="r_rot", bufs=2)
        for fc in range(4):
            ps_o = psX.tile([P, 512], F32, name="ps_o", tag="ps", bufs=8)
            for kp in range(CP):
                nc.tensor.matmul(ps_o[:], yT[:, 2 * kp:2 * kp + 2, qt * P:(qt + 1) * P],
                                 wo_t[:, 2 * kp:2 * kp + 2, fc, :],
                                 start=(kp == 0), stop=False, perf_mode=DR)
            nc.tensor.matmul(ps_o[:], ones1[:], bo_bc[:, fc * 512:(fc + 1) * 512],
                             start=False, stop=True)
            r_sl = r_rot[:, fc * 512:(fc + 1) * 512]
            nc.vector.tensor_tensor(r_sl, ps_o[:], xq_t[:, fc * 512:(fc + 1) * 512],
                                    OP.add)
        nc.sync.dma_start(r_d[qt * P:(qt + 1) * P, :], r_rot[:])
        xn2_t = _ln_tile(nc, pD, r_rot[:], eps_t, "ln2", qt)
        nc.sync.dma_start_transpose(xn2T[:, :, qt * P:(qt + 1) * P], xn2_t[:])
    pD.release()

    # ---------------- Stage F: MLP up + gelu -> hT [128, FT, R] bf16
    pF = tc.alloc_tile_pool(name="pF", bufs=2)
    for fp in range(FT // 2):
        w1c = pF.tile([P, CT, 2, P], BF16, name="w1c", tag="w1c", bufs=3)
        nc.scalar.dma_start(w1c[:], d["w1"][fp])
        for fl in range(2):
            fo = 2 * fp + fl
            ps_h = psX.tile([P, R], F32, name="ps_h", tag="ps", bufs=8)
            for ki in range(CT):
                nc.tensor.matmul(ps_h[:], w1c[:, ki, fl, :], xn2T[:, ki, :],
                                 start=(ki == 0), stop=(ki == CT - 1))
            nc.scalar.activation(out=hT[:, fo, :], in_=ps_h[:], func=ACT.Gelu,
                                 bias=b1_t[:, fo:fo + 1], scale=1.0)
    pF.release()
    p_xn2T.release()

    # ---------------- Stage G: MLP down + residual -> out
    pG = tc.alloc_tile_pool(name="pG", bufs=2)
    for fc in range(4):
        ps4 = [psX.tile([P, 512], F32, name=f"ps_g{qt}", tag="ps", bufs=8)
               for qt in range(RT)]
        rgs = []
        for qt in range(RT):
            r_g = pG.tile([P, 512], F32, name="r_g", tag="r_g", bufs=8)
            nc.sync.dma_start(r_g[:], r_d[qt * P:(qt + 1) * P,
                                          fc * 512:(fc + 1) * 512])
            rgs.append(r_g)
        for hb in range(FT // 8):
            w2b = pG.tile([P, 8, 512], BF16, name="w2b", tag="w2b", bufs=3)
            nc.scalar.dma_start(w2b[:], d["w2"][fc, hb])
            for hl in range(8):
                ho = hb * 8 + hl
                for qt in range(RT):
                    nc.tensor.matmul(ps4[qt][:], hT[:, ho, qt * P:(qt + 1) * P],
                                     w2b[:, hl, :], start=(ho == 0), stop=False)
        for qt in range(RT):
            nc.tensor.matmul(ps4[qt][:], ones1[:], b2_bc[:, fc * 512:(fc + 1) * 512],
                             start=False, stop=True)
            o_t = pG.tile([P, 512], F32, name="o_t", tag="o_t", bufs=3)
            nc.vector.tensor_tensor(o_t[:], ps4[qt][:], rgs[qt][:], OP.add)
            nc.sync.dma_start(d["out"][qt * P:(qt + 1) * P, fc * 512:(fc + 1) * 512], o_t[:])
    pG.release()
    psX.release()
    p_hT.release()
    p_wo.release()
    p_yT.release()
    p_rd.release()
    consts.release()


def build_program():
    nc = bacc.Bacc("TRN2", target_bir_lowering=False, debug=False, num_devices=8)
    with tile.TileContext(nc) as tc:
        _body(tc)
    nc.compile()
    return nc


_prog = None


def _get_prog():
    global _prog
    if _prog is None:
        _prog = build_program()
    return _prog


def make_in_maps(x, mask, Wq, bq, Wk, bk, Wv, bv, Wo, bo,
                 ln1_w, ln1_b, ln2_w, ln2_b, W1, b1, W2, b2):
    bf = ml_dtypes.bfloat16
    f8 = ml_dtypes.float8_e4m3
    f32 = np.float32
    cc = np.ascontiguousarray

    def f(a):
        return np.asarray(a, dtype=f32)

    x, mask = np.asarray(x, dtype=f32), np.asarray(mask)
    # fold the LN affines into the consuming matmuls: for y = ln(x)@W + b with
    # ln(x) = z*w + b_ln (z the normalized input), y = z@(w[:,None]*W) + (b_ln@W + b)
    w1l, b1l = f(ln1_w)[:, None], f(ln1_b)
    w2l, b2l = f(ln2_w)[:, None], f(ln2_b)
    Wq_, Wk_, Wv_, W1_ = w1l * f(Wq), w1l * f(Wk), w1l * f(Wv), w2l * f(W1)
    bq_, bk_ = f(bq) + b1l @ f(Wq), f(bk) + b1l @ f(Wk)
    bv_, b1_ = f(bv) + b1l @ f(Wv), f(b1) + b2l @ f(W1)
    wq_h = cc(Wq_.astype(f8).reshape(CT, P, CT, P).transpose(2, 1, 0, 3))
    wk_h = cc(Wk_.astype(f8).reshape(CT, P, CT, P).transpose(2, 1, 0, 3))
    wv_h = cc(Wv_.astype(f8).reshape(CT, P, C).transpose(1, 0, 2))
    wo_h = cc(f(Wo).astype(f8).reshape(CT, P, 4, 512).transpose(1, 0, 2, 3))
    w1_h = cc(W1_.astype(bf).reshape(CT, P, FT // 2, 2, P).transpose(2, 1, 0, 3, 4))
    w2_h = cc(f(W2).astype(bf).reshape(FT // 8, 8, P, 4, 512).transpose(3, 0, 2, 1, 4))
    shared = dict(
        wq=wq_h, wk=wk_h, wv=wv_h, wo=wo_h, w1=w1_h, w2=w2_h,
        bq=cc(bq_.reshape(CT, P).T), bk=cc(bk_.reshape(CT, P).T),
        b1=cc(b1_.reshape(FT, P).T),
        bv=bv_.astype(bf), bo=f(bo).astype(bf), b2=f(b2).astype(bf),
    )
    in_maps = []
    for c in range(8):
        b, hh = divmod(c, 2)
        xc = np.roll(x[b], -hh * R, axis=0)
        mk = np.roll(np.asarray(mask[b, hh * R:(hh + 1) * R, :], dtype=f32),
                     -hh * R, axis=1).astype(bf)
        in_maps.append({**shared, "xb": cc(xc.astype(bf)), "xq": cc(xc[:R]),
                        "mask": cc(mk)})
    return in_maps


def kernel(**inputs):
    nc = _get_prog()
    in_maps = make_in_maps(**inputs)
    res = run_bass_kernel_spmd(nc, in_maps, core_ids=list(range(8)))
    out = np.empty((B, T, C), np.float32)
    for c in range(8):
        b, hh = divmod(c, 2)
        out[b, hh * R:(hh + 1) * R, :] = res.results[c]["out"]
    return out


# revision 25
# speedup vs baseline: 1.0563x; 1.0009x over previous
"""Transformer block (LN -> MHA -> residual -> LN -> MLP -> residual) on 8 TRN2
NeuronCores.

Sharding: pure row data-parallelism over (batch, sequence-half). Core c handles
batch b = c//2 and query rows [h*512, (h+1)*512) with h = c%2. Each core
computes K/V projections for its full batch locally (small duplicated work),
which removes every cross-core collective. Host reorders each core's batch rows
"own rows first" so the same SPMD program works on all cores; mask columns are
permuted identically (softmax/attention are permutation-invariant over keys).

v2: overlap restructure + fp8 attention.
  - All attention matmuls (Q/K/V proj, scores, AV, O proj) run in fp8-e4m3
    with perf_mode=DoubleRow (two 128-row contraction tiles per issue),
    ~1.8x the bf16 matmul rate. MLP matmuls stay bf16 (fp8 there blows the
    2e-2 error budget; attention-fp8 costs ~1.3e-2, verified by simulation).
  - LN1 runs per-row-tile; V projections for a token tile are emitted right
    after its transpose, so the PE has work ~10us in instead of idling ~80us.
  - softmax drops the max-subtraction: logits are bounded (|q.k|*isq <~ 15,
    masked lanes are -30000 -> exp==0), so exp never overflows fp32/bf16.
  - LN2 is fused into the O-projection loop per query tile; the attention
    residual r stays in SBUF (no DRAM bounce).
  - output stores stream per (qt, fc) chunk.
fp32 PSUM accumulation everywhere; statistics stay fp32.
"""

import numpy as np
import ml_dtypes

import concourse.bass as bass
import concourse.tile as tile
from concourse import bacc, mybir
from concourse.bass_utils import run_bass_kernel_spmd

BF16 = mybir.dt.bfloat16
F32 = mybir.dt.float32
FP8 = mybir.dt.float8e4
AX = mybir.AxisListType
OP = mybir.AluOpType
ACT = mybir.ActivationFunctionType
DR = mybir.MatmulPerfMode.DoubleRow

P = 128
B, T, C, H = 4, 1024, 2048, 4
DH = C // H                      # 512
F = 4 * C                        # 8192
R = T // 2                       # 512 own query rows per core
RT, TT, CT, FT = R // P, T // P, C // P, F // P   # 4, 8, 16, 64
CP = CT // 2                     # 8 double-row contraction steps over C
HT = DH // P                     # 4 feature tiles per head
EPS = 1e-5
ISQ = 1.0 / float(np.sqrt(DH))
NEGBIG = 30000.0


def _bcast_load(nc, pool, dram_ap, name, dtype):
    """Broadcast a [n] DRAM vector to all 128 partitions -> [128, n]."""
    t = pool.tile([P, dram_ap.shape[0]], dtype, name=name)
    src = bass.AP(
        tensor=dram_ap.tensor, offset=dram_ap.offset, ap=[[0, P]] + list(dram_ap.ap)
    )
    nc.gpsimd.dma_start(out=t[:], in_=src)
    return t


def _ln_tile(nc, pool, x_sl, eps_t, tag, i):
    """Normalize one [128, C] tile -> bf16 (x-mu)*rstd. The LN affine (w,b)
    is folded into the following matmul's weights/biases on the host."""
    stats = pool.tile([P, 4, 6], F32, name=f"{tag}_stats{i}", tag=f"{tag}_stats",
                      bufs=2)
    for sg in range(4):
        nc.vector.bn_stats(out=stats[:, sg, :], in_=x_sl[:, sg * 512:(sg + 1) * 512])
    mv = pool.tile([P, 2], F32, name=f"{tag}_mv{i}", tag=f"{tag}_mv", bufs=2)
    nc.vector.bn_aggr(out=mv[:], in_=stats[:])
    std = pool.tile([P, 1], F32, name=f"{tag}_std{i}", tag=f"{tag}_std", bufs=2)
    nc.scalar.activation(out=std[:], in_=mv[:, 1:2], func=ACT.Sqrt,
                         bias=eps_t[:], scale=1.0)
    rstd = pool.tile([P, 1], F32, name=f"{tag}_rstd{i}", tag=f"{tag}_rstd", bufs=2)
    nc.vector.reciprocal(rstd[:], std[:])
    nmr = pool.tile([P, 1], F32, name=f"{tag}_nmr{i}", tag=f"{tag}_nmr", bufs=2)
    nc.vector.tensor_scalar(nmr[:], mv[:, 0:1], rstd[:], -1.0, OP.mult, OP.mult)
    xh = pool.tile([P, C], BF16, name=f"{tag}_xh{i}", tag=f"{tag}_xh", bufs=2)
    nc.scalar.activation(out=xh[:], in_=x_sl, func=ACT.Identity,
                         bias=nmr[:], scale=rstd[:])
    return xh


def _body(tc):
    nc = tc.nc
    d = {n: nc.dram_tensor(n, s, dt, kind=k).ap() for n, s, dt, k in [
        ("xb", [T, C], BF16, "ExternalInput"),
        ("xq", [R, C], F32, "ExternalInput"),
        ("mask", [R, T], BF16, "ExternalInput"),
        ("wq", [CT, P, CT, P], FP8, "ExternalInput"),
        ("wk", [CT, P, CT, P], FP8, "ExternalInput"),
        ("wv", [P, CT, C], FP8, "ExternalInput"),
        ("wo", [P, CT, 4, 512], FP8, "ExternalInput"),
        ("w1", [FT // 2, P, CT, 2, P], BF16, "ExternalInput"),
        ("w2", [4, FT // 8, P, 8, 512], BF16, "ExternalInput"),
        ("bq", [P, CT], F32, "ExternalInput"),
        ("bk", [P, CT], F32, "ExternalInput"),
        ("b1", [P, FT], F32, "ExternalInput"),
        ("bv", [C], BF16, "ExternalInput"),
        ("bo", [C], BF16, "ExternalInput"),
        ("b2", [C], BF16, "ExternalInput"),
        ("out", [R, C], F32, "ExternalOutput"),
    ]}

    consts = tc.alloc_tile_pool(name="consts", bufs=1)
    eps_t = consts.tile([P, 1], F32, name="eps")
    nc.vector.memset(eps_t[:], EPS)
    p_rd = tc.alloc_tile_pool(name="p_rd", bufs=1, space="DRAM")
    r_d = p_rd.tile([R, C], F32, name="r_d")
    # long-lived SBUF tensors (allocated first: the pool stack is LIFO)
    p_yT = tc.alloc_tile_pool(name="p_yT", bufs=1)
    yT = p_yT.tile([P, CT, R], FP8, name="yT")
    p_wo = tc.alloc_tile_pool(name="p_wo", bufs=1)
    wo_t = p_wo.tile([P, CT, 4, 512], FP8, name="wo_t")
    p_xnT = tc.alloc_tile_pool(name="p_xnT", bufs=1)
    xnT8_lo = p_xnT.tile([P, CT, R], FP8, name="xnT8_lo")
    xnT8_hi = p_xnT.tile([P, CT, R], FP8, name="xnT8_hi")
    xnT8 = [xnT8_lo, xnT8_hi]
    p_vh = tc.alloc_tile_pool(name="p_vh", bufs=1)
    vh = p_vh.tile([P, TT, C], FP8, name="vh")

    # ---------------- Stage A: per-tile LN1 -> transpose -> fp8 cast -> V proj
    p_wv = tc.alloc_tile_pool(name="p_wv", bufs=1)
    wv_t = p_wv.tile([P, CT, C], FP8, name="wv_t")
    lnA = tc.alloc_tile_pool(name="lnA", bufs=1)
    bv_bc = _bcast_load(nc, lnA, d["bv"], "bv_bc", BF16)
    pA = tc.alloc_tile_pool(name="pA", bufs=2)
    psA = tc.alloc_tile_pool(name="psA", bufs=2, space="PSUM")

    # x tiles stream on the gpsimd queue; wv (needed in full by the first
    # V matmul group) streams on the scalar queue in parallel.
    xts = []
    for tt in range(TT):
        xt = pA.tile([P, C], BF16, name=f"xt{tt}", tag="xt", bufs=3)
        nc.sync.dma_start(xt[:], d["xb"][tt * P:(tt + 1) * P, :])
        xts.append(xt[:])
    for kc in range(2):
        nc.scalar.dma_start(wv_t[:, 4 * kc:4 * (kc + 1), :],
                            d["wv"][:, 4 * kc:4 * (kc + 1), :])

    # software-pipelined by one tile: cast(tt)+V(tt) are emitted during
    # LN(tt+1) so the fp8 cast's transpose-wait never blocks the next LN
    # apply in the scalar FIFO.
    xnTts = {}

    def _emit_castv(tt):
        half, lt = divmod(tt, 4)
        nc.vector.tensor_copy(xnT8[half][:, :, lt * P:(lt + 1) * P],
                              xnTts.pop(tt)[:])
        for h in range(H):
            ps_v = psA.tile([P, DH], F32, name="ps_v", tag="psA", bufs=2)
            for kp in range(CP):
                nc.tensor.matmul(ps_v[:],
                                 xnT8[half][:, 2 * kp:2 * kp + 2, lt * P:(lt + 1) * P],
                                 wv_t[:, 2 * kp:2 * kp + 2, h * DH:(h + 1) * DH],
                                 start=(kp == 0), stop=(kp == CP - 1), perf_mode=DR)
            nc.vector.tensor_tensor(vh[:, tt, h * DH:(h + 1) * DH], ps_v[:],
                                    bv_bc[:, h * DH:(h + 1) * DH], OP.add)

    for tt in range(TT):
        xn_t = _ln_tile(nc, pA, xts[tt], eps_t, "ln1", tt)
        xnTt = pA.tile([P, CT, P], BF16, name=f"xnTt{tt}", tag="xnTt", bufs=3)
        nc.scalar.dma_start_transpose(xnTt[:], xn_t[:])
        xnTts[tt] = xnTt
        if tt < 2:  # interleave the rest of wv behind the first transposes
            nc.scalar.dma_start(wv_t[:, 4 * (tt + 2):4 * (tt + 3), :],
                                d["wv"][:, 4 * (tt + 2):4 * (tt + 3), :])
        if tt > 0:
            _emit_castv(tt - 1)
    _emit_castv(TT - 1)
    psA.release()
    pA.release()
    lnA.release()
    p_wv.release()

    # remaining constants (emitted after stage A so their DMAs don't delay it)
    bo_bc = _bcast_load(nc, consts, d["bo"], "bo_bc", BF16)
    b2_bc = _bcast_load(nc, consts, d["b2"], "b2_bc", BF16)
    bq_t = consts.tile([P, CT], F32, name="bq_t")
    nc.gpsimd.dma_start(out=bq_t[:], in_=d["bq"])
    bk_t = consts.tile([P, CT], F32, name="bk_t")
    nc.gpsimd.dma_start(out=bk_t[:], in_=d["bk"])
    b1_t = consts.tile([P, FT], F32, name="b1_t")
    nc.gpsimd.dma_start(out=b1_t[:], in_=d["b1"])
    # mask -> additive bias: 0 where visible, -30000 where masked
    p_mb = tc.alloc_tile_pool(name="p_mb", bufs=1)
    mb = p_mb.tile([P, RT, T], BF16, name="mb")
    nc.gpsimd.dma_start(out=mb[:], in_=d["mask"].rearrange("(qo qp) k -> qp qo k", qp=P))
    nc.vector.tensor_scalar(mb[:], mb[:], NEGBIG, -NEGBIG, OP.mult, OP.add)

    # ---------------- Stage B+C: software-pipelined per-head Q/K + attention
    pBC = tc.alloc_tile_pool(name="pBC", bufs=2)
    psBC = tc.alloc_tile_pool(name="psBC", bufs=2, space="PSUM")
    hs = {}

    def emit_qk(h):
        qTh = pBC.tile([P, HT, R], FP8, name=f"qTh{h}", tag="qTh", bufs=2)
        kTh = pBC.tile([P, HT, T], FP8, name=f"kTh{h}", tag="kTh", bufs=2)
        eng = nc.sync if h % 2 == 0 else nc.scalar
        wqcs, wkcs = [], []
        for fl in range(HT):
            fo = h * HT + fl
            wqc = pBC.tile([P, CT, P], FP8, name="wqc", tag="wqc", bufs=4)
            eng.dma_start(wqc[:], d["wq"][fo])
            wqcs.append(wqc)
            wkc = pBC.tile([P, CT, P], FP8, name="wkc", tag="wkc", bufs=4)
            eng.dma_start(wkc[:], d["wk"][fo])
            wkcs.append(wkc)
        for fl in range(HT):
            fo = h * HT + fl
            wqc, wkc = wqcs[fl], wkcs[fl]
            ps_q = psBC.tile([P, R], F32, name="ps_q", tag="psB", bufs=2)
            for kp in range(CP):
                nc.tensor.matmul(ps_q[:], wqc[:, 2 * kp:2 * kp + 2, :],
                                 xnT8_lo[:, 2 * kp:2 * kp + 2, :],
                                 start=(kp == 0), stop=(kp == CP - 1), perf_mode=DR)
            nc.scalar.activation(out=qTh[:, fl, :], in_=ps_q[:], func=ACT.Identity,
                                 bias=bq_t[:, fo:fo + 1], scale=1.0)
            for nn in range(2):
                ps_k = psBC.tile([P, 512], F32, name="ps_k", tag="psB", bufs=2)
                for kp in range(CP):
                    nc.tensor.matmul(ps_k[:], wkc[:, 2 * kp:2 * kp + 2, :],
                                     xnT8[nn][:, 2 * kp:2 * kp + 2, :],
                                     start=(kp == 0), stop=(kp == CP - 1), perf_mode=DR)
                nc.scalar.activation(out=kTh[:, fl, nn * 512:(nn + 1) * 512], in_=ps_k[:],
                                     func=ACT.Identity, bias=bk_t[:, fo:fo + 1],
                                     scale=1.0)
        hs[h] = (qTh, kTh)

    def emit_scores(h):
        qTh, kTh = hs[h]
        attT = pBC.tile([P, TT, R], BF16, name=f"attT{h}", tag="attT", bufs=2)
        attT8 = pBC.tile([P, TT, R], FP8, name=f"attT8{h}", tag="attT8", bufs=2)
        for qt in range(RT):
            ps_s = psBC.tile([P, T], F32, name="ps_s", tag="scores", bufs=2)
            for nn in range(2):
                for dp in range(HT // 2):
                    nc.tensor.matmul(
                        ps_s[:, nn * 512:(nn + 1) * 512],
                        qTh[:, 2 * dp:2 * dp + 2, qt * P:(qt + 1) * P],
                        kTh[:, 2 * dp:2 * dp + 2, nn * 512:(nn + 1) * 512],
                        start=(dp == 0), stop=(dp == HT // 2 - 1), perf_mode=DR)
            s_sb = pBC.tile([P, T], F32, name="s_sb", tag="s_sb", bufs=2)
            nc.vector.scalar_tensor_tensor(s_sb[:], ps_s[:], ISQ, mb[:, qt, :],
                                           OP.mult, OP.add)
            # logits are bounded (<= ~15) so exp needs no max-subtraction
            e_sb = pBC.tile([P, T], BF16, name="e_sb", tag="e_sb", bufs=2)
            sums = pBC.tile([P, 1], F32, name="sums", tag="sums", bufs=2)
            nc.scalar.activation(out=e_sb[:], in_=s_sb[:], func=ACT.Exp,
                                 bias=0.0, scale=1.0, accum_out=sums[:])
            recip = pBC.tile([P, 1], F32, name="recip", tag="recip", bufs=2)
            nc.vector.reciprocal(recip[:], sums[:])
            nc.vector.tensor_scalar_mul(e_sb[:], e_sb[:], recip[:])
            nc.sync.dma_start_transpose(attT[:, :, qt * P:(qt + 1) * P], e_sb[:])
        if h == H - 1:  # per-qt cast so the split AV can start immediately
            for qt in range(RT):
                nc.vector.tensor_copy(attT8[:, :, qt * P:(qt + 1) * P],
                                      attT[:, :, qt * P:(qt + 1) * P])
        else:
            nc.vector.tensor_copy(attT8[:], attT[:])
        hs[h] = hs[h] + (attT8,)

    def emit_av(h):
        _, _, attT8 = hs.pop(h)
        nq = RT if h == H - 1 else 1   # last head: split over qt chunks so AV
        nw = R // nq                   # overlaps the tail softmax chain
        for dt_ in range(HT):
            ps_y = psBC.tile([P, R], F32, name="ps_y", tag="av", bufs=2)
            for qc in range(nq):
                for kp in range(TT // 2):
                    nc.tensor.matmul(
                        ps_y[:, qc * nw:(qc + 1) * nw],
                        vh[:, 2 * kp:2 * kp + 2, h * DH + dt_ * P:h * DH + (dt_ + 1) * P],
                        attT8[:, 2 * kp:2 * kp + 2, qc * nw:(qc + 1) * nw],
                        start=(kp == 0), stop=(kp == TT // 2 - 1), perf_mode=DR)
            nc.scalar.activation(out=yT[:, h * HT + dt_, :], in_=ps_y[:], func=ACT.Copy)

    emit_qk(0)
    emit_scores(0)
    # preload Wo (4 MB fp8) while attention runs
    for kc in range(4):
        nc.scalar.dma_start(wo_t[:, 4 * kc:4 * (kc + 1), :, :],
                            d["wo"][:, 4 * kc:4 * (kc + 1), :, :])
    for h in range(H):
        if h + 1 < H:
            emit_qk(h + 1)
        emit_av(h)
        if h + 1 < H:
            emit_scores(h + 1)
    psBC.release()
    pBC.release()
    p_mb.release()
    p_vh.release()
    p_xnT.release()

    # ones1 has a single 1 in row 0: ones1.T @ bias_bc adds a bias chunk
    # inside the matmul accumulation (used in stages D and G).
    ones1 = consts.tile([P, P], BF16, name="ones1")
    nc.vector.memset(ones1[:], 0.0)
    nc.vector.memset(ones1[0:1, :], 1.0)

    # ---------------- Stage D: per-qt O-proj + residual + LN2
    # r rows rotate through SBUF (feeding LN2) and bounce to DRAM for stage G.
    # One shared PSUM pool (psX) serves D, F and G so no stage-boundary
    # write-after-read stall on fresh PSUM banks.
    p_hT = tc.alloc_tile_pool(name="p_hT", bufs=1)
    hT = p_hT.tile([P, FT, R], BF16, name="hT")
    psX = tc.alloc_tile_pool(name="psX", bufs=8, space="PSUM")
    p_xn2T = tc.alloc_tile_pool(name="p_xn2T", bufs=1)
    xn2T = p_xn2T.tile([P, CT, R], BF16, name="xn2T")
    pF = tc.alloc_tile_pool(name="pF", bufs=2)
    w1cs = {}
    for fp in range(3):   # prefetch the first w1 chunks during stage D
        w1c = pF.tile([P, CT, 2, P], BF16, name="w1c", tag="w1c", bufs=3)
        nc.scalar.dma_start(w1c[:], d["w1"][fp])
        w1cs[fp] = w1c
    pD = tc.alloc_tile_pool(name="pD", bufs=2)
    xqs = []
    for qt in range(RT):
        xq_t = pD.tile([P, C], F32, name=f"xq{qt}", tag="xq_t", bufs=3)
        nc.sync.dma_start(xq_t[:], d["xq"][qt * P:(qt + 1) * P, :])
        xqs.append(xq_t)
    for qt in range(RT):
        xq_t = xqs[qt]
        r_rot = pD.tile([P, C], F32, name=f"r{qt}", tag="r_rot", bufs=2)
        for fc in range(4):
            ps_o = psX.tile([P, 512], F32, name="ps_o", tag="ps", bufs=8)
            for kp in range(CP):
                nc.tensor.matmul(ps_o[:], yT[:, 2 * kp:2 * kp + 2, qt * P:(qt + 1) * P],
                                 wo_t[:, 2 * kp:2 * kp + 2, fc, :],
                                 start=(kp == 0), stop=False, perf_mode=DR)
            nc.tensor.matmul(ps_o[:], ones1[:], bo_bc[:, fc * 512:(fc + 1) * 512],
                             start=False, stop=True)
            r_sl = r_rot[:, fc * 512:(fc + 1) * 512]
            nc.vector.tensor_tensor(r_sl, ps_o[:], xq_t[:, fc * 512:(fc + 1) * 512],
                                    OP.add)
        nc.sync.dma_start(r_d[qt * P:(qt + 1) * P, :], r_rot[:])
        xn2_t = _ln_tile(nc, pD, r_rot[:], eps_t, "ln2", qt)
        nc.sync.dma_start_transpose(xn2T[:, :, qt * P:(qt + 1) * P], xn2_t[:])
    pD.release()

    # ---------------- Stage F: MLP up + gelu -> hT [128, FT, R] bf16
    for fp in range(FT // 2):
        if fp in w1cs:
            w1c = w1cs.pop(fp)
        else:
            w1c = pF.tile([P, CT, 2, P], BF16, name="w1c", tag="w1c", bufs=3)
            nc.scalar.dma_start(w1c[:], d["w1"][fp])
        for fl in range(2):
            fo = 2 * fp + fl
            ps_h = psX.tile([P, R], F32, name="ps_h", tag="ps", bufs=8)
            for ki in range(CT):
                nc.tensor.matmul(ps_h[:], w1c[:, ki, fl, :], xn2T[:, ki, :],
                                 start=(ki == 0), stop=(ki == CT - 1))
            nc.scalar.activation(out=hT[:, fo, :], in_=ps_h[:], func=ACT.Gelu,
                                 bias=b1_t[:, fo:fo + 1], scale=1.0)
    pF.release()
    p_xn2T.release()

    # ---------------- Stage G: MLP down + residual -> out
    pG = tc.alloc_tile_pool(name="pG", bufs=2)
    for fc in range(4):
        ps4 = [psX.tile([P, 512], F32, name=f"ps_g{qt}", tag="ps", bufs=8)
               for qt in range(RT)]
        rgs = []
        for qt in range(RT):
            r_g = pG.tile([P, 512], F32, name="r_g", tag="r_g", bufs=8)
            nc.scalar.dma_start(r_g[:], r_d[qt * P:(qt + 1) * P,
                                            fc * 512:(fc + 1) * 512])
            rgs.append(r_g)
        for hb in range(FT // 8):
            w2b = pG.tile([P, 8, 512], BF16, name="w2b", tag="w2b", bufs=3)
            nc.sync.dma_start(w2b[:], d["w2"][fc, hb])
            for hl in range(8):
                ho = hb * 8 + hl
                for qt in range(RT):
                    nc.tensor.matmul(ps4[qt][:], hT[:, ho, qt * P:(qt + 1) * P],
                                     w2b[:, hl, :], start=(ho == 0), stop=False)
        for qt in range(RT):
            nc.tensor.matmul(ps4[qt][:], ones1[:], b2_bc[:, fc * 512:(fc + 1) * 512],
                             start=False, stop=True)
            o_t = pG.tile([P, 512], F32, name="o_t", tag="o_t", bufs=3)
            nc.vector.tensor_tensor(o_t[:], ps4[qt][:], rgs[qt][:], OP.add)
            nc.scalar.dma_start(d["out"][qt * P:(qt + 1) * P, fc * 512:(fc + 1) * 512],
                                o_t[:])
    pG.release()
    psX.release()
    p_hT.release()
    p_wo.release()
    p_yT.release()
    p_rd.release()
    consts.release()


def build_program():
    nc = bacc.Bacc("TRN2", target_bir_lowering=False, debug=False, num_devices=8)
    with tile.TileContext(nc) as tc:
        _body(tc)
    nc.compile()
    return nc


_prog = None


def _get_prog():
    global _prog
    if _prog is None:
        _prog = build_program()
    return _prog


def make_in_maps(x, mask, Wq, bq, Wk, bk, Wv, bv, Wo, bo,
                 ln1_w, ln1_b, ln2_w, ln2_b, W1, b1, W2, b2):
    bf = ml_dtypes.bfloat16
    f8 = ml_dtypes.float8_e4m3
    f32 = np.float32
    cc = np.ascontiguousarray

    def f(a):
        return np.asarray(a, dtype=f32)

    x, mask = np.asarray(x, dtype=f32), np.asarray(mask)
    # fold the LN affines into the consuming matmuls: for y = ln(x)@W + b with
    # ln(x) = z*w + b_ln (z the normalized input), y = z@(w[:,None]*W) + (b_ln@W + b)
    w1l, b1l = f(ln1_w)[:, None], f(ln1_b)
    w2l, b2l = f(ln2_w)[:, None], f(ln2_b)
    Wq_, Wk_, Wv_, W1_ = w1l * f(Wq), w1l * f(Wk), w1l * f(Wv), w2l * f(W1)
    bq_, bk_ = f(bq) + b1l @ f(Wq), f(bk) + b1l @ f(Wk)
    bv_, b1_ = f(bv) + b1l @ f(Wv), f(b1) + b2l @ f(W1)
    wq_h = cc(Wq_.astype(f8).reshape(CT, P, CT, P).transpose(2, 1, 0, 3))
    wk_h = cc(Wk_.astype(f8).reshape(CT, P, CT, P).transpose(2, 1, 0, 3))
    wv_h = cc(Wv_.astype(f8).reshape(CT, P, C).transpose(1, 0, 2))
    wo_h = cc(f(Wo).astype(f8).reshape(CT, P, 4, 512).transpose(1, 0, 2, 3))
    w1_h = cc(W1_.astype(bf).reshape(CT, P, FT // 2, 2, P).transpose(2, 1, 0, 3, 4))
    w2_h = cc(f(W2).astype(bf).reshape(FT // 8, 8, P, 4, 512).transpose(3, 0, 2, 1, 4))
    shared = dict(
        wq=wq_h, wk=wk_h, wv=wv_h, wo=wo_h, w1=w1_h, w2=w2_h,
        bq=cc(bq_.reshape(CT, P).T), bk=cc(bk_.reshape(CT, P).T),
        b1=cc(b1_.reshape(FT, P).T),
        bv=bv_.astype(bf), bo=f(bo).astype(bf), b2=f(b2).astype(bf),
    )
    in_maps = []
    for c in range(8):
        b, hh = divmod(c, 2)
        xc = np.roll(x[b], -hh * R, axis=0)
        mk = np.roll(np.asarray(mask[b, hh * R:(hh + 1) * R, :], dtype=f32),
                     -hh * R, axis=1).astype(bf)
        in_maps.append({**shared, "xb": cc(xc.astype(bf)), "xq": cc(xc[:R]),
                        "mask": cc(mk)})
    return in_maps


def kernel(**inputs):
    nc = _get_prog()
    in_maps = make_in_maps(**inputs)
    res = run_bass_kernel_spmd(nc, in_maps, core_ids=list(range(8)))
    out = np.empty((B, T, C), np.float32)
    for c in range(8):
        b, hh = divmod(c, 2)
        out[b, hh * R:(hh + 1) * R, :] = res.results[c]["out"]
    return out


# revision 28
# speedup vs baseline: 1.0691x; 1.0121x over previous
"""Transformer block (LN -> MHA -> residual -> LN -> MLP -> residual) on 8 TRN2
NeuronCores.

Sharding: pure row data-parallelism over (batch, sequence-half). Core c handles
batch b = c//2 and query rows [h*512, (h+1)*512) with h = c%2. Each core
computes K/V projections for its full batch locally (small duplicated work),
which removes every cross-core collective. Host reorders each core's batch rows
"own rows first" so the same SPMD program works on all cores; mask columns are
permuted identically (softmax/attention are permutation-invariant over keys).

v2: overlap restructure + fp8 attention.
  - All attention matmuls (Q/K/V proj, scores, AV, O proj) run in fp8-e4m3
    with perf_mode=DoubleRow (two 128-row contraction tiles per issue),
    ~1.8x the bf16 matmul rate. MLP matmuls stay bf16 (fp8 there blows the
    2e-2 error budget; attention-fp8 costs ~1.3e-2, verified by simulation).
  - LN1 runs per-row-tile; V projections for a token tile are emitted right
    after its transpose, so the PE has work ~10us in instead of idling ~80us.
  - softmax drops the max-subtraction: logits are bounded (|q.k|*isq <~ 15,
    masked lanes are -30000 -> exp==0), so exp never overflows fp32/bf16.
  - LN2 is fused into the O-projection loop per query tile; the attention
    residual r stays in SBUF (no DRAM bounce).
  - output stores stream per (qt, fc) chunk.
fp32 PSUM accumulation everywhere; statistics stay fp32.
"""

import numpy as np
import ml_dtypes

import concourse.bass as bass
import concourse.tile as tile
from concourse import bacc, mybir
from concourse.bass_utils import run_bass_kernel_spmd

BF16 = mybir.dt.bfloat16
F32 = mybir.dt.float32
FP8 = mybir.dt.float8e4
AX = mybir.AxisListType
OP = mybir.AluOpType
ACT = mybir.ActivationFunctionType
DR = mybir.MatmulPerfMode.DoubleRow

P = 128
B, T, C, H = 4, 1024, 2048, 4
DH = C // H                      # 512
F = 4 * C                        # 8192
R = T // 2                       # 512 own query rows per core
RT, TT, CT, FT = R // P, T // P, C // P, F // P   # 4, 8, 16, 64
CP = CT // 2                     # 8 double-row contraction steps over C
HT = DH // P                     # 4 feature tiles per head
EPS = 1e-5
ISQ = 1.0 / float(np.sqrt(DH))
NEGBIG = 30000.0


def _bcast_load(nc, pool, dram_ap, name, dtype):
    """Broadcast a [n] DRAM vector to all 128 partitions -> [128, n]."""
    t = pool.tile([P, dram_ap.shape[0]], dtype, name=name)
    src = bass.AP(
        tensor=dram_ap.tensor, offset=dram_ap.offset, ap=[[0, P]] + list(dram_ap.ap)
    )
    nc.gpsimd.dma_start(out=t[:], in_=src)
    return t


def _ln_tile(nc, pool, x_sl, eps_t, tag, i):
    """Normalize one [128, C] tile -> bf16 (x-mu)*rstd. The LN affine (w,b)
    is folded into the following matmul's weights/biases on the host."""
    stats = pool.tile([P, 4, 6], F32, name=f"{tag}_stats{i}", tag=f"{tag}_stats",
                      bufs=2)
    for sg in range(4):
        nc.vector.bn_stats(out=stats[:, sg, :], in_=x_sl[:, sg * 512:(sg + 1) * 512])
    mv = pool.tile([P, 2], F32, name=f"{tag}_mv{i}", tag=f"{tag}_mv", bufs=2)
    nc.vector.bn_aggr(out=mv[:], in_=stats[:])
    std = pool.tile([P, 1], F32, name=f"{tag}_std{i}", tag=f"{tag}_std", bufs=2)
    nc.scalar.activation(out=std[:], in_=mv[:, 1:2], func=ACT.Sqrt,
                         bias=eps_t[:], scale=1.0)
    rstd = pool.tile([P, 1], F32, name=f"{tag}_rstd{i}", tag=f"{tag}_rstd", bufs=2)
    nc.vector.reciprocal(rstd[:], std[:])
    nmr = pool.tile([P, 1], F32, name=f"{tag}_nmr{i}", tag=f"{tag}_nmr", bufs=2)
    nc.vector.tensor_scalar(nmr[:], mv[:, 0:1], rstd[:], -1.0, OP.mult, OP.mult)
    xh = pool.tile([P, C], BF16, name=f"{tag}_xh{i}", tag=f"{tag}_xh", bufs=2)
    nc.scalar.activation(out=xh[:], in_=x_sl, func=ACT.Identity,
                         bias=nmr[:], scale=rstd[:])
    return xh


def _body(tc):
    nc = tc.nc
    d = {n: nc.dram_tensor(n, s, dt, kind=k).ap() for n, s, dt, k in [
        ("xb", [T, C], BF16, "ExternalInput"),
        ("xq", [R, C], F32, "ExternalInput"),
        ("mask", [R, T], BF16, "ExternalInput"),
        ("wq", [CT, P, CT, P], FP8, "ExternalInput"),
        ("wk", [CT, P, CT, P], FP8, "ExternalInput"),
        ("wv", [P, CT, C], FP8, "ExternalInput"),
        ("wo", [P, CT, 4, 512], FP8, "ExternalInput"),
        ("w1", [FT // 2, P, CT, 2, P], BF16, "ExternalInput"),
        ("w2", [4, FT // 8, P, 8, 512], BF16, "ExternalInput"),
        ("bq", [P, CT], F32, "ExternalInput"),
        ("bk", [P, CT], F32, "ExternalInput"),
        ("b1", [P, FT], F32, "ExternalInput"),
        ("bv", [C], BF16, "ExternalInput"),
        ("bo", [C], BF16, "ExternalInput"),
        ("b2", [C], BF16, "ExternalInput"),
        ("out", [R, C], F32, "ExternalOutput"),
    ]}

    consts = tc.alloc_tile_pool(name="consts", bufs=1)
    eps_t = consts.tile([P, 1], F32, name="eps")
    nc.vector.memset(eps_t[:], EPS)
    p_rd = tc.alloc_tile_pool(name="p_rd", bufs=1, space="DRAM")
    r_d = p_rd.tile([R, C], F32, name="r_d")
    # long-lived SBUF tensors (allocated first: the pool stack is LIFO)
    p_yT = tc.alloc_tile_pool(name="p_yT", bufs=1)
    yT = p_yT.tile([P, CT, R], FP8, name="yT")
    p_wo = tc.alloc_tile_pool(name="p_wo", bufs=1)
    wo_t = p_wo.tile([P, CT, 4, 512], FP8, name="wo_t")
    p_xnT = tc.alloc_tile_pool(name="p_xnT", bufs=1)
    xnT8_lo = p_xnT.tile([P, CT, R], FP8, name="xnT8_lo")
    xnT8_hi = p_xnT.tile([P, CT, R], FP8, name="xnT8_hi")
    xnT8 = [xnT8_lo, xnT8_hi]
    p_vh = tc.alloc_tile_pool(name="p_vh", bufs=1)
    vh = p_vh.tile([P, TT, C], FP8, name="vh")

    # ---------------- Stage A: per-tile LN1 -> transpose -> fp8 cast -> V proj
    p_wv = tc.alloc_tile_pool(name="p_wv", bufs=1)
    wv_t = p_wv.tile([P, CT, C], FP8, name="wv_t")
    lnA = tc.alloc_tile_pool(name="lnA", bufs=1)
    bv_bc = _bcast_load(nc, lnA, d["bv"], "bv_bc", BF16)
    pA = tc.alloc_tile_pool(name="pA", bufs=2)
    psA = tc.alloc_tile_pool(name="psA", bufs=2, space="PSUM")

    # x tiles stream on the gpsimd queue; wv (needed in full by the first
    # V matmul group) streams on the scalar queue in parallel.
    xts = []
    for tt in range(TT):
        xt = pA.tile([P, C], BF16, name=f"xt{tt}", tag="xt", bufs=3)
        nc.sync.dma_start(xt[:], d["xb"][tt * P:(tt + 1) * P, :])
        xts.append(xt[:])
    for kc in range(2):
        nc.scalar.dma_start(wv_t[:, 4 * kc:4 * (kc + 1), :],
                            d["wv"][:, 4 * kc:4 * (kc + 1), :])

    # software-pipelined by one tile: cast(tt)+V(tt) are emitted during
    # LN(tt+1) so the fp8 cast's transpose-wait never blocks the next LN
    # apply in the scalar FIFO.
    xnTts = {}

    def _emit_castv(tt):
        half, lt = divmod(tt, 4)
        nc.vector.tensor_copy(xnT8[half][:, :, lt * P:(lt + 1) * P],
                              xnTts.pop(tt)[:])
        for h in range(H):
            ps_v = psA.tile([P, DH], F32, name="ps_v", tag="psA", bufs=2)
            for kp in range(CP):
                nc.tensor.matmul(ps_v[:],
                                 xnT8[half][:, 2 * kp:2 * kp + 2, lt * P:(lt + 1) * P],
                                 wv_t[:, 2 * kp:2 * kp + 2, h * DH:(h + 1) * DH],
                                 start=(kp == 0), stop=(kp == CP - 1), perf_mode=DR)
            nc.vector.tensor_tensor(vh[:, tt, h * DH:(h + 1) * DH], ps_v[:],
                                    bv_bc[:, h * DH:(h + 1) * DH], OP.add)

    for tt in range(TT):
        xn_t = _ln_tile(nc, pA, xts[tt], eps_t, "ln1", tt)
        xnTt = pA.tile([P, CT, P], BF16, name=f"xnTt{tt}", tag="xnTt", bufs=3)
        nc.scalar.dma_start_transpose(xnTt[:], xn_t[:])
        xnTts[tt] = xnTt
        if tt < 2:  # interleave the rest of wv behind the first transposes
            nc.scalar.dma_start(wv_t[:, 4 * (tt + 2):4 * (tt + 3), :],
                                d["wv"][:, 4 * (tt + 2):4 * (tt + 3), :])
        if tt > 0:
            _emit_castv(tt - 1)
    _emit_castv(TT - 1)
    psA.release()
    pA.release()
    lnA.release()
    p_wv.release()

    # remaining constants (emitted after stage A so their DMAs don't delay it)
    bo_bc = _bcast_load(nc, consts, d["bo"], "bo_bc", BF16)
    b2_bc = _bcast_load(nc, consts, d["b2"], "b2_bc", BF16)
    bq_t = consts.tile([P, CT], F32, name="bq_t")
    nc.gpsimd.dma_start(out=bq_t[:], in_=d["bq"])
    bk_t = consts.tile([P, CT], F32, name="bk_t")
    nc.gpsimd.dma_start(out=bk_t[:], in_=d["bk"])
    b1_t = consts.tile([P, FT], F32, name="b1_t")
    nc.gpsimd.dma_start(out=b1_t[:], in_=d["b1"])
    # mask -> additive bias: 0 where visible, -30000 where masked
    p_mb = tc.alloc_tile_pool(name="p_mb", bufs=1)
    mb = p_mb.tile([P, RT, T], BF16, name="mb")
    nc.gpsimd.dma_start(out=mb[:], in_=d["mask"].rearrange("(qo qp) k -> qp qo k", qp=P))
    nc.vector.tensor_scalar(mb[:], mb[:], NEGBIG, -NEGBIG, OP.mult, OP.add)

    # ---------------- Stage B+C: software-pipelined per-head Q/K + attention
    pBC = tc.alloc_tile_pool(name="pBC", bufs=2)
    psBC = tc.alloc_tile_pool(name="psBC", bufs=2, space="PSUM")
    hs = {}

    def emit_qk(h):
        qTh = pBC.tile([P, HT, R], FP8, name=f"qTh{h}", tag="qTh", bufs=2)
        kTh = pBC.tile([P, HT, T], FP8, name=f"kTh{h}", tag="kTh", bufs=2)
        eng = nc.sync if h % 2 == 0 else nc.scalar
        wqcs, wkcs = [], []
        for fl in range(HT):
            fo = h * HT + fl
            wqc = pBC.tile([P, CT, P], FP8, name="wqc", tag="wqc", bufs=4)
            eng.dma_start(wqc[:], d["wq"][fo])
            wqcs.append(wqc)
            wkc = pBC.tile([P, CT, P], FP8, name="wkc", tag="wkc", bufs=4)
            eng.dma_start(wkc[:], d["wk"][fo])
            wkcs.append(wkc)
        for fl in range(HT):
            fo = h * HT + fl
            wqc, wkc = wqcs[fl], wkcs[fl]
            ps_q = psBC.tile([P, R], F32, name="ps_q", tag="psB", bufs=2)
            for kp in range(CP):
                nc.tensor.matmul(ps_q[:], wqc[:, 2 * kp:2 * kp + 2, :],
                                 xnT8_lo[:, 2 * kp:2 * kp + 2, :],
                                 start=(kp == 0), stop=(kp == CP - 1), perf_mode=DR)
            nc.scalar.activation(out=qTh[:, fl, :], in_=ps_q[:], func=ACT.Identity,
                                 bias=bq_t[:, fo:fo + 1], scale=1.0)
            for nn in range(2):
                ps_k = psBC.tile([P, 512], F32, name="ps_k", tag="psB", bufs=2)
                for kp in range(CP):
                    nc.tensor.matmul(ps_k[:], wkc[:, 2 * kp:2 * kp + 2, :],
                                     xnT8[nn][:, 2 * kp:2 * kp + 2, :],
                                     start=(kp == 0), stop=(kp == CP - 1), perf_mode=DR)
                nc.scalar.activation(out=kTh[:, fl, nn * 512:(nn + 1) * 512], in_=ps_k[:],
                                     func=ACT.Identity, bias=bk_t[:, fo:fo + 1],
                                     scale=1.0)
        hs[h] = (qTh, kTh)

    def emit_scores(h):
        qTh, kTh = hs[h]
        attT = pBC.tile([P, TT, R], BF16, name=f"attT{h}", tag="attT", bufs=2)
        attT8 = pBC.tile([P, TT, R], FP8, name=f"attT8{h}", tag="attT8", bufs=2)
        for qt in range(RT):
            ps_s = psBC.tile([P, T], F32, name="ps_s", tag="scores", bufs=2)
            for nn in range(2):
                for dp in range(HT // 2):
                    nc.tensor.matmul(
                        ps_s[:, nn * 512:(nn + 1) * 512],
                        qTh[:, 2 * dp:2 * dp + 2, qt * P:(qt + 1) * P],
                        kTh[:, 2 * dp:2 * dp + 2, nn * 512:(nn + 1) * 512],
                        start=(dp == 0), stop=(dp == HT // 2 - 1), perf_mode=DR)
            s_sb = pBC.tile([P, T], F32, name="s_sb", tag="s_sb", bufs=2)
            nc.vector.scalar_tensor_tensor(s_sb[:], ps_s[:], ISQ, mb[:, qt, :],
                                           OP.mult, OP.add)
            # logits are bounded (<= ~15) so exp needs no max-subtraction
            e_sb = pBC.tile([P, T], BF16, name="e_sb", tag="e_sb", bufs=2)
            sums = pBC.tile([P, 1], F32, name="sums", tag="sums", bufs=2)
            nc.scalar.activation(out=e_sb[:], in_=s_sb[:], func=ACT.Exp,
                                 bias=0.0, scale=1.0, accum_out=sums[:])
            recip = pBC.tile([P, 1], F32, name="recip", tag="recip", bufs=2)
            nc.vector.reciprocal(recip[:], sums[:])
            nc.vector.tensor_scalar_mul(e_sb[:], e_sb[:], recip[:])
            nc.sync.dma_start_transpose(attT[:, :, qt * P:(qt + 1) * P], e_sb[:])
        if h == H - 1:  # per-qt cast so the split AV can start immediately
            for qt in range(RT):
                nc.vector.tensor_copy(attT8[:, :, qt * P:(qt + 1) * P],
                                      attT[:, :, qt * P:(qt + 1) * P])
        else:
            nc.vector.tensor_copy(attT8[:], attT[:])
        hs[h] = hs[h] + (attT8,)

    def emit_av(h):
        _, _, attT8 = hs.pop(h)
        nq = RT if h == H - 1 else 1   # last head: split over qt chunks so AV
        nw = R // nq                   # overlaps the tail softmax chain
        for dt_ in range(HT):
            ps_y = psBC.tile([P, R], F32, name="ps_y", tag="av", bufs=2)
            for qc in range(nq):
                for kp in range(TT // 2):
                    nc.tensor.matmul(
                        ps_y[:, qc * nw:(qc + 1) * nw],
                        vh[:, 2 * kp:2 * kp + 2, h * DH + dt_ * P:h * DH + (dt_ + 1) * P],
                        attT8[:, 2 * kp:2 * kp + 2, qc * nw:(qc + 1) * nw],
                        start=(kp == 0), stop=(kp == TT // 2 - 1), perf_mode=DR)
            nc.scalar.activation(out=yT[:, h * HT + dt_, :], in_=ps_y[:], func=ACT.Copy)

    emit_qk(0)
    emit_scores(0)
    # preload Wo (4 MB fp8) while attention runs
    for kc in range(4):
        nc.scalar.dma_start(wo_t[:, 4 * kc:4 * (kc + 1), :, :],
                            d["wo"][:, 4 * kc:4 * (kc + 1), :, :])
    for h in range(H):
        if h + 1 < H:
            emit_qk(h + 1)
        emit_av(h)
        if h + 1 < H:
            emit_scores(h + 1)
    psBC.release()
    pBC.release()
    p_mb.release()
    p_vh.release()
    p_xnT.release()

    # ones1 has a single 1 in row 0: ones1.T @ bias_bc adds a bias chunk
    # inside the matmul accumulation (used in stages D and G).
    ones1 = consts.tile([P, P], BF16, name="ones1")
    nc.vector.memset(ones1[:], 0.0)
    nc.vector.memset(ones1[0:1, :], 1.0)

    # ---------------- Stage D: per-qt O-proj + residual + LN2
    # r rows rotate through SBUF (feeding LN2) and bounce to DRAM for stage G.
    # One shared PSUM pool (psX) serves D, F and G so no stage-boundary
    # write-after-read stall on fresh PSUM banks.
    p_hT = tc.alloc_tile_pool(name="p_hT", bufs=1)
    hT = p_hT.tile([P, FT, R], BF16, name="hT")
    psX = tc.alloc_tile_pool(name="psX", bufs=8, space="PSUM")
    p_xn2T = tc.alloc_tile_pool(name="p_xn2T", bufs=1)
    xn2T = p_xn2T.tile([P, CT, R], BF16, name="xn2T")
    pF = tc.alloc_tile_pool(name="pF", bufs=2)
    w1cs = {}
    for fp in range(3):   # prefetch the first w1 chunks during stage D
        w1c = pF.tile([P, CT, 2, P], BF16, name="w1c", tag="w1c", bufs=3)
        nc.scalar.dma_start(w1c[:], d["w1"][fp])
        w1cs[fp] = w1c
    pD = tc.alloc_tile_pool(name="pD", bufs=2)
    xqs = []
    for qt in range(RT):
        xq_t = pD.tile([P, C], F32, name=f"xq{qt}", tag="xq_t", bufs=3)
        nc.sync.dma_start(xq_t[:], d["xq"][qt * P:(qt + 1) * P, :])
        xqs.append(xq_t)
    for qt in range(RT):
        xq_t = xqs[qt]
        r_rot = pD.tile([P, C], F32, name=f"r{qt}", tag="r_rot", bufs=2)
        for fc in range(4):
            ps_o = psX.tile([P, 512], F32, name="ps_o", tag="ps", bufs=8)
            for kp in range(CP):
                nc.tensor.matmul(ps_o[:], yT[:, 2 * kp:2 * kp + 2, qt * P:(qt + 1) * P],
                                 wo_t[:, 2 * kp:2 * kp + 2, fc, :],
                                 start=(kp == 0), stop=False, perf_mode=DR)
            nc.tensor.matmul(ps_o[:], ones1[:], bo_bc[:, fc * 512:(fc + 1) * 512],
                             start=False, stop=True)
            r_sl = r_rot[:, fc * 512:(fc + 1) * 512]
            nc.vector.tensor_tensor(r_sl, ps_o[:], xq_t[:, fc * 512:(fc + 1) * 512],
                                    OP.add)
        nc.sync.dma_start(r_d[qt * P:(qt + 1) * P, :], r_rot[:])
        xn2_t = _ln_tile(nc, pD, r_rot[:], eps_t, "ln2", qt)
        nc.sync.dma_start_transpose(xn2T[:, :, qt * P:(qt + 1) * P], xn2_t[:])
    pD.release()

    # ---------------- Stage F: MLP up + gelu -> hT [128, FT, R] bf16
    for fp in range(FT // 2):
        if fp in w1cs:
            w1c = w1cs.pop(fp)
        else:
            w1c = pF.tile([P, CT, 2, P], BF16, name="w1c", tag="w1c", bufs=3)
            nc.scalar.dma_start(w1c[:], d["w1"][fp])
        for fl in range(2):
            fo = 2 * fp + fl
            ps_h = psX.tile([P, R], F32, name="ps_h", tag="ps", bufs=8)
            for ki in range(CT):
                nc.tensor.matmul(ps_h[:], w1c[:, ki, fl, :], xn2T[:, ki, :],
                                 start=(ki == 0), stop=(ki == CT - 1))
            nc.scalar.activation(out=hT[:, fo, :], in_=ps_h[:], func=ACT.Gelu,
                                 bias=b1_t[:, fo:fo + 1], scale=1.0)
    pF.release()
    p_xn2T.release()

    # ---------------- Stage G: MLP down + residual -> out
    pG = tc.alloc_tile_pool(name="pG", bufs=2)
    for fc in range(4):
        ps4 = [psX.tile([P, 512], F32, name=f"ps_g{qt}", tag="ps", bufs=8)
               for qt in range(RT)]
        rgs = []
        for qt in range(RT):
            r_g = pG.tile([P, 512], F32, name="r_g", tag="r_g", bufs=8)
            nc.scalar.dma_start(r_g[:], r_d[qt * P:(qt + 1) * P,
                                            fc * 512:(fc + 1) * 512])
            rgs.append(r_g)
        for hb in range(FT // 8):
            w2b = pG.tile([P, 8, 512], BF16, name="w2b", tag="w2b", bufs=3)
            nc.sync.dma_start(w2b[:], d["w2"][fc, hb])
            for hl in range(8):
                ho = hb * 8 + hl
                for qt in range(RT):
                    nc.tensor.matmul(ps4[qt][:], hT[:, ho, qt * P:(qt + 1) * P],
                                     w2b[:, hl, :], start=(ho == 0), stop=False)
        for qt in range(RT):
            nc.tensor.matmul(ps4[qt][:], ones1[:], b2_bc[:, fc * 512:(fc + 1) * 512],
                             start=False, stop=True)
            o_t = pG.tile([P, 512], F32, name="o_t", tag="o_t", bufs=3)
            nc.vector.tensor_tensor(o_t[:], ps4[qt][:], rgs[qt][:], OP.add)
            nc.scalar.dma_start(d["out"][qt * P:(qt + 1) * P, fc * 512:(fc + 1) * 512],
                                o_t[:])
    pG.release()
    psX.release()
    p_hT.release()
    p_wo.release()
    p_yT.release()
    p_rd.release()
    consts.release()


def build_program():
    nc = bacc.Bacc("TRN2", target_bir_lowering=False, debug=False, num_devices=8)
    with tile.TileContext(nc) as tc:
        _body(tc)
    nc.compile()
    return nc


_prog = None


def _get_prog():
    global _prog
    if _prog is None:
        _prog = build_program()
    return _prog


def make_in_maps(x, mask, Wq, bq, Wk, bk, Wv, bv, Wo, bo,
                 ln1_w, ln1_b, ln2_w, ln2_b, W1, b1, W2, b2):
    bf = ml_dtypes.bfloat16
    f8 = ml_dtypes.float8_e4m3
    f32 = np.float32
    cc = np.ascontiguousarray

    def f(a):
        return np.asarray(a, dtype=f32)

    x, mask = np.asarray(x, dtype=f32), np.asarray(mask)
    # fold the LN affines into the consuming matmuls: for y = ln(x)@W + b with
    # ln(x) = z*w + b_ln (z the normalized input), y = z@(w[:,None]*W) + (b_ln@W + b)
    w1l, b1l = f(ln1_w)[:, None], f(ln1_b)
    w2l, b2l = f(ln2_w)[:, None], f(ln2_b)
    Wq_, Wk_, Wv_, W1_ = w1l * f(Wq), w1l * f(Wk), w1l * f(Wv), w2l * f(W1)
    bq_, bk_ = f(bq) + b1l @ f(Wq), f(bk) + b1l @ f(Wk)
    bv_, b1_ = f(bv) + b1l @ f(Wv), f(b1) + b2l @ f(W1)
    wq_h = cc(Wq_.astype(f8).reshape(CT, P, CT, P).transpose(2, 1, 0, 3))
    wk_h = cc(Wk_.astype(f8).reshape(CT, P, CT, P).transpose(2, 1, 0, 3))
    wv_h = cc(Wv_.astype(f8).reshape(CT, P, C).transpose(1, 0, 2))
    wo_h = cc(f(Wo).astype(f8).reshape(CT, P, 4, 512).transpose(1, 0, 2, 3))
    w1_h = cc(W1_.astype(bf).reshape(CT, P, FT // 2, 2, P).transpose(2, 1, 0, 3, 4))
    w2_h = cc(f(W2).astype(bf).reshape(FT // 8, 8, P, 4, 512).transpose(3, 0, 2, 1, 4))
    shared = dict(
        wq=wq_h, wk=wk_h, wv=wv_h, wo=wo_h, w1=w1_h, w2=w2_h,
        bq=cc(bq_.reshape(CT, P).T), bk=cc(bk_.reshape(CT, P).T),
        b1=cc(b1_.reshape(FT, P).T),
        bv=bv_.astype(bf), bo=f(bo).astype(bf), b2=f(b2).astype(bf),
    )
    in_maps = []
    for c in range(8):
        b, hh = divmod(c, 2)
        xc = np.roll(x[b], -hh * R, axis=0)
        mk = np.roll(np.asarray(mask[b, hh * R:(hh + 1) * R, :], dtype=f32),
                     -hh * R, axis=1).astype(bf)
        in_maps.append({**shared, "xb": cc(xc.astype(bf)), "xq": cc(xc[:R]),
                        "mask": cc(mk)})
    return in_maps


def kernel(**inputs):
    nc = _get_prog()
    in_maps = make_in_maps(**inputs)
    res = run_bass_kernel_spmd(nc, in_maps, core_ids=list(range(8)))
    out = np.empty((B, T, C), np.float32)
    for c in range(8):
        b, hh = divmod(c, 2)
        out[b, hh * R:(hh + 1) * R, :] = res.results[c]["out"]
    return out
